# revision 24
# baseline (speedup 1.0000x reference)
"""Trainium2 Bass kernel for LogitBiasedSelfAttention1D.

Sharding: 8 cores = (batch b in 0..3) x (query half qh in 0..1).
Each core computes full attention (all 8 heads, all 2048 keys) for its
1024 queries of its batch. No collectives.

Math decomposition (exactly equivalent to the reference up to fp):
  - conv1d key bias folded into V:  softmax(S + bias) @ V
      = (exp(S) @ (c * V)) / (exp(S) @ c),   c = exp(bias)
  - SCALE folded into w_q on host.
  - residual x_seq + b_out added via identity matmul (f32r) into the
    out_proj PSUM accumulation.
  - LayerNorm normalize runs on ScalarE (Identity with scale/bias APs).

Schedule: the exp softmax on ScalarE is the bottleneck (~141us busy), so
everything is pipelined around keeping it fed:
  - QKV projections are emitted as small psum-chunk units interleaved into
    head-pair 0's S-loop (separate PSUM tags), so exp starts ~6us in.
  - PV (attn @ V) for pair p runs interleaved inside pair p+1's S-loop
    (PSUM accumulator banks conflict with projection psum at the head, so
    PV lags one pair; exp outputs buffer in a deep pt pool).
  - The LN tail is per-t-chunk pipelined across ACT/PE/DVE with the sqrt
    table preloaded during pair-3 PV, and the output DMA split in quarters.
"""

import sys

for _p in ("/opt/trn_rl_repo", "/root/.axon_site/_ro/trn_rl_repo"):
    if _p not in sys.path:
        sys.path.insert(0, _p)

import numpy as np
import ml_dtypes

from concourse import bass, mybir
from concourse.tile import TileContext
from concourse.bass_utils import run_bass_kernel_spmd

B, C, T = 4, 512, 2048
H, D = 8, 64
SCALE = D ** -0.5
EPS = 1e-5
TQ = T // 2            # queries per core
KC = T // 128          # 16 key chunks
PAIRS = H // 2         # 4 head pairs
F32 = mybir.dt.float32
F32R = mybir.dt.float32r
BF16 = mybir.dt.bfloat16
bf16 = ml_dtypes.bfloat16

Exp = mybir.ActivationFunctionType.Exp
SCH_A = float(2**7 / np.log(2))    # Schraudolph fast-exp, bf16 bits in int16
SCH_B = float(127 * 2**7 - 7.5)
Sqrt = mybir.ActivationFunctionType.Sqrt
Ident = mybir.ActivationFunctionType.Identity
MULT = mybir.AluOpType.mult
ADD = mybir.AluOpType.add

_CACHE = {}


def _build_nc(reps=1, split_waits=True):
    nc = bass.Bass()
    xctm = nc.declare_dram_parameter("xctm", [128, 4 * T], BF16, False)
    xqm = nc.declare_dram_parameter("xqm", [128, 4 * TQ], BF16, False)
    xsm = nc.declare_dram_parameter("xsm", [128, 8 * C], F32R, False)
    wqm = nc.declare_dram_parameter("wqm", [128, 4 * C], BF16, False)
    wkm = nc.declare_dram_parameter("wkm", [128, 4 * C], BF16, False)
    wvm = nc.declare_dram_parameter("wvm", [128, 4 * C], BF16, False)
    wom = nc.declare_dram_parameter("wom", [128, 4 * C], BF16, False)
    cful = nc.declare_dram_parameter("cful", [128, KC], F32, False)
    c8 = nc.declare_dram_parameter("c8", [128, KC * H], BF16, False)
    gmm = nc.declare_dram_parameter("gmm", [128, 4], F32, False)
    bet = nc.declare_dram_parameter("bet", [128, 4], F32, False)
    iden = nc.declare_dram_parameter("iden", [128, 128], BF16, False)
    idenr = nc.declare_dram_parameter("idenr", [128, 128], F32R, False)
    outp = nc.declare_dram_parameter("out", [128, 4 * TQ], BF16, True)

    with TileContext(nc) as tc:
        with (
            tc.sbuf_pool(name="cst", bufs=1) as cst,
            tc.sbuf_pool(name="pex", bufs=20) as pex,
            tc.sbuf_pool(name="sml", bufs=2) as sml,
            tc.psum_pool(name="ps", bufs=1) as ps,
        ):
            # ---- persistent state tiles ----
            epsT = cst.tile([128, 1], F32, name="epsT")
            dummy = cst.tile([128, 1], F32, name="dummy")
            KT = [cst.tile([128, T], BF16, name=f"KT{m}") for m in range(4)]
            QT = [cst.tile([128, TQ], BF16, name=f"QT{m}") for m in range(4)]
            VB = [cst.tile([128, H * 65], BF16, name=f"VB{k}") for k in range(KC)]
            OT = [cst.tile([128, TQ], BF16, name=f"OTp{p}") for p in range(PAIRS)]
            OUTS = cst.tile([128, 4 * TQ], BF16, name="OUTS")
            MV = cst.tile([128, 16], F32, name="MV")       # (mean, var) x 8 t
            RSD = cst.tile([128, 16], F32, name="RSD")     # rstd8 | -mu*rstd

            # preload the exp table while input DMAs run
            nc.vector.memset(epsT[:, :], EPS)
            nc.scalar.activation(dummy[:, :], epsT[:, :], Exp)

            # ---- input loads, compute-critical first ----
            WQb = cst.tile_from(wqm[:, :], name="WQb")
            XQb = cst.tile_from(xqm[:, :], name="XQb",
                                forced_dma_engine=mybir.EngineType.Pool)
            WKb = cst.tile_from(wkm[:, :], name="WKb")
            XCTb = cst.tile_from(xctm[:, :], name="XCTb",
                                 forced_dma_engine=mybir.EngineType.Pool)
            WVb = cst.tile_from(wvm[:, :], name="WVb")
            CF = cst.tile_from(cful[:, :], name="CF")
            C8 = cst.tile_from(c8[:, :], name="C8")
            ID = cst.tile_from(iden[:, :], name="ID")
            WOb = cst.tile_from(wom[:, :], name="WOb")
            IDR = cst.tile_from(idenr[:, :], name="IDR")
            XSb = cst.tile_from(xsm[:, :], name="XSb",
                                forced_dma_engine=mybir.EngineType.Pool)
            GM = cst.tile_from(gmm[:, :], name="GM")
            BT = cst.tile_from(bet[:, :], name="BT")
            WQ = [WQb[:, i * C:(i + 1) * C] for i in range(4)]
            XQ = [XQb[:, i * TQ:(i + 1) * TQ] for i in range(4)]
            WK = [WKb[:, i * C:(i + 1) * C] for i in range(4)]
            XCT = [XCTb[:, i * T:(i + 1) * T] for i in range(4)]
            WV = [WVb[:, i * C:(i + 1) * C] for i in range(4)]
            WO = [WOb[:, i * C:(i + 1) * C] for i in range(4)]
            XS = [XSb[:, t * C:(t + 1) * C] for t in range(8)]

            def _body(rp):
                # ---------- projection units (psum chunk each) ----------
                pcnt = [0]

                def q_unit(m, n):
                    qps = ps.tile([128, 512], F32,
                                  tag=("OA1", "OA2")[pcnt[0] % 2],
                                  name=f"qps{m}_{n}_r{rp}")
                    pcnt[0] += 1
                    for ci in range(4):
                        nc.tensor.matmul(
                            qps[:, :],
                            lhsT=WQ[ci][:, m * 128:(m + 1) * 128],
                            rhs=XQ[ci][:, n * 512:(n + 1) * 512],
                            start=(ci == 0), stop=(ci == 3))
                    nc.vector.tensor_copy(
                        QT[m][:, n * 512:(n + 1) * 512], qps[:, :])

                def k_unit(m, h2, n):
                    kps = ps.tile([128, 512], F32,
                                  tag=("OA1", "OA2")[pcnt[0] % 2],
                                  name=f"kps{m}_{h2}_{n}_r{rp}")
                    pcnt[0] += 1
                    for ci in range(4):
                        nc.tensor.matmul(
                            kps[:, :],
                            lhsT=WK[ci][:, m * 128:(m + 1) * 128],
                            rhs=XCT[ci][:, h2 * 1024 + n * 512:
                                        h2 * 1024 + (n + 1) * 512],
                            start=(ci == 0), stop=(ci == 3))
                    nc.vector.tensor_copy(
                        KT[m][:, h2 * 1024 + n * 512:h2 * 1024 + (n + 1) * 512],
                        kps[:, :])

                def v_unit(k):
                    vps = ps.tile([128, 512], F32, tag=("OB1", "OB2")[k % 2],
                                  name=f"vps{k}_r{rp}")
                    for ci in range(4):
                        nc.tensor.matmul(
                            vps[:, :],
                            lhsT=XCT[ci][:, k * 128:(k + 1) * 128],
                            rhs=WV[ci][:, :],
                            start=(ci == 0), stop=(ci == 3))
                    nc.vector.tensor_scalar(
                        out=VB[k].rearrange("p (h e) -> p h e", e=65)[:, :, 0:64],
                        in0=vps.rearrange("p (h e) -> p h e", e=64),
                        scalar1=CF[:, k:k + 1], scalar2=None, op0=MULT)
                    nc.vector.tensor_copy(
                        VB[k].rearrange("p (h e) -> p h e", e=65)[:, :, 64:65],
                        C8[:, k * H:(k + 1) * H].rearrange("p (h e) -> p h e", e=1))

                units = []
                units += [lambda n=n: k_unit(0, 1, n) for n in range(2)]
                units += [lambda n=n: q_unit(1, n) for n in range(2)]
                units += [lambda a=a: k_unit(1, *a)
                          for a in ((0, 0), (0, 1), (1, 0), (1, 1))]
                units += [lambda k=k: v_unit(k) for k in range(KC)]
                units += [lambda n=n: q_unit(2, n) for n in range(2)]
                units += [lambda a=a: k_unit(2, *a)
                          for a in ((0, 0), (0, 1), (1, 0), (1, 1))]
                units += [lambda n=n: q_unit(3, n) for n in range(2)]
                units += [lambda a=a: k_unit(3, *a)
                          for a in ((0, 0), (0, 1), (1, 0), (1, 1))]
                uidx = [0]

                def emit_units(cnt):
                    for _ in range(cnt):
                        if uidx[0] < len(units):
                            units[uidx[0]]()
                            uidx[0] += 1

                # upfront: QT0 fully, KT0 first half (covers S k<8)
                q_unit(0, 0); q_unit(0, 1); k_unit(0, 0, 0); k_unit(0, 0, 1)

                PT = {}    # (p, k, hi) -> pt tile, buffered until PV consumes
                last_pt = [None]

                def s_exp(p, k, hi, on_dve=False):
                    stag, ptag = ("SA", "pA") if hi == 0 else ("SB", "pB")
                    rows = slice(hi * 64, (hi + 1) * 64)
                    s_ps = ps.tile([128, 1024], F32, tag=stag,
                                   name=f"s{p}_{hi}_{k}_r{rp}")
                    for n in range(2):
                        nc.tensor.matmul(
                            s_ps[:, n * 512:(n + 1) * 512],
                            lhsT=KT[p][rows, k * 128:(k + 1) * 128],
                            rhs=QT[p][rows, n * 512:(n + 1) * 512],
                            start=True, stop=True)
                    pt = pex.tile([128, 1024], BF16, tag=ptag,
                                  name=f"pt{p}_{hi}_{k}_r{rp}")
                    if on_dve:
                        nc.vector.tensor_scalar(
                            out=pt.bitcast(mybir.dt.int16)[:, :], in0=s_ps[:, :],
                            scalar1=SCH_A, scalar2=SCH_B, op0=MULT, op1=ADD)
                    else:
                        nc.scalar.activation(pt[:, :], s_ps[:, :], Exp)
                        last_pt[0] = pt
                    PT[(p, k, hi)] = pt

                OACC = {}  # p -> {hi: [2 psum tiles]}

                def pv_half(p, k, hi):
                    if p not in OACC:
                        OACC[p] = {
                            0: [ps.tile([128, 512], F32, tag=("OA1", "OA2")[bk],
                                        name=f"o{p}_0_{bk}_r{rp}")
                                for bk in range(2)],
                            1: [ps.tile([128, 512], F32, tag=("OB1", "OB2")[bk],
                                        name=f"o{p}_1_{bk}_r{rp}")
                                for bk in range(2)],
                        }
                    pt = PT.pop((p, k, hi))
                    head = 2 * p + hi
                    for s in range(8):
                        bk, j = s // 4, s % 4
                        nc.tensor.matmul(
                            OACC[p][hi][bk][:, j * 65:(j + 1) * 65],
                            lhsT=pt[:, s * 128:(s + 1) * 128],
                            rhs=VB[k][:, head * 65:(head + 1) * 65],
                            start=(k == 0), stop=(k == KC - 1),
                            skip_group_check=True)

                def pv_chunk(p, k):
                    pv_half(p, k, 0)
                    pv_half(p, k, 1)

                def epilogue(p, use_act=False):
                    oacc = OACC.pop(p)
                    nmul = 0
                    ONs = [sml.tile([128, 128], BF16, tag="on",
                                    name=f"on{p}_{s}_r{rp}", bufs=10)
                           for s in range(8)]
                    for hi in (0, 1):
                        for bk in range(2):
                            o_ps = oacc[hi][bk]
                            ov = o_ps[:, 0:260].rearrange("p (s e) -> p s e", e=65)
                            rd4 = sml.tile([128, 4], F32, tag="rd",
                                           name=f"rd{p}_{hi}_{bk}_r{rp}", bufs=4)
                            nc.vector.reciprocal(
                                rd4.rearrange("p (s e) -> p s e", e=1),
                                ov[:, :, 64:65])
                            for j in range(4):
                                s = bk * 4 + j
                                nmul += 1
                                if use_act and nmul > 10:
                                    nc.scalar.activation(
                                        ONs[s][:, hi * 64:(hi + 1) * 64],
                                        o_ps[:, j * 65:j * 65 + 64],
                                        Ident, scale=rd4[:, j:j + 1])
                                else:
                                    nc.vector.tensor_scalar_mul(
                                        ONs[s][:, hi * 64:(hi + 1) * 64],
                                        o_ps[:, j * 65:j * 65 + 64],
                                        rd4[:, j:j + 1])
                    tps = [ps.tile([128, 512], BF16, tag=("OA1", "OA2")[b_],
                                   name=f"tp{p}_{b_}_r{rp}")
                           for b_ in range(2)]
                    for s in range(8):
                        nc.tensor.transpose(
                            tps[s // 4][:, (s % 4) * 128:(s % 4 + 1) * 128],
                            ONs[s][:, :], ID[:, :])
                    for b_ in range(2):
                        nc.vector.tensor_copy(OT[p][:, b_ * 512:(b_ + 1) * 512],
                                              tps[b_][:, :])

                # ---------- pair loops: S/exp for p, PV for p-1 ----------
                for p in range(PAIRS):
                    for k in range(KC):
                        s_exp(p, k, 0, on_dve=(p > 0 and k % 2 == 1))
                        if p > 0 and k > 0:
                            pv_half(p - 1, k - 1, 0)
                        s_exp(p, k, 1, on_dve=(p > 0 and k % 2 == 0 and k > 0))
                        if p == 0:
                            emit_units(3)
                        elif k > 0:
                            pv_half(p - 1, k - 1, 1)
                    emit_units(99)  # flush leftovers (end of pair 0 only)
                    if p > 0:
                        pv_chunk(p - 1, KC - 1)
                        epilogue(p - 1)

                # preload sqrt table right after the last exp (the read of
                # last_pt pins this behind the final Exp so the scheduler
                # cannot hoist it ahead and thrash the exp table)
                nc.scalar.activation(dummy[:, :], last_pt[0][:, 0:1], Sqrt)

                # ---------- PV(3) + early out_proj partials (t<4) ----------
                opsT = [None] * 8
                bigs = []
                for i in range(2):
                    big = ps.tile([128, 1024], F32, tag=("SA", "SB")[i],
                                  name=f"opb{i}_r{rp}")
                    bigs.append(big)
                    opsT[2 * i] = big[:, 0:512]
                    opsT[2 * i + 1] = big[:, 512:1024]
                for k in range(KC):
                    pv_chunk(3, k)
                    if k < 4:
                        t = k
                        for p_ in range(3):
                            nc.tensor.matmul(
                                opsT[t][:, :],
                                lhsT=OT[p_][:, t * 128:(t + 1) * 128],
                                rhs=WO[p_][:, :],
                                start=(p_ == 0), stop=False)
                epilogue(3, use_act=True)

                # ---------- finish out_proj + residual + LN stats ----------
                for t in range(8):
                    if t >= 4:
                        opsT[t] = ps.tile([128, 512], F32,
                                          tag=("OA1", "OA2", "OB1", "OB2")[t - 4],
                                          name=f"op{t}_r{rp}")
                        for p_ in range(4):
                            nc.tensor.matmul(
                                opsT[t][:, :],
                                lhsT=OT[p_][:, t * 128:(t + 1) * 128],
                                rhs=WO[p_][:, :],
                                start=(p_ == 0), stop=False)
                    else:
                        nc.tensor.matmul(
                            opsT[t][:, :],
                            lhsT=OT[3][:, t * 128:(t + 1) * 128],
                            rhs=WO[3][:, :],
                            start=False, stop=False)
                    nc.tensor.matmul(
                        opsT[t][:, :], lhsT=IDR[:, :], rhs=XS[t][:, :],
                        start=False, stop=True)
                    bnst = sml.tile([128, 6], F32, tag="bnst",
                                    name=f"bnst{t}_r{rp}", bufs=3)
                    nc.vector.bn_stats(bnst[:, :], opsT[t][:, :])
                    nc.vector.bn_aggr(MV[:, 2 * t:2 * t + 2], bnst[:, :])

                # ---------- per-t pipelined LN + transpose + store ----------
                HN = [sml.tile([128, C], BF16, tag="hn", name=f"hn{t}_r{rp}",
                               bufs=8) for t in range(8)]
                std = [sml.tile([128, 1], F32, tag="std", name=f"std{t}_r{rp}",
                                bufs=4) for t in range(8)]

                def ln_t(t):
                    nc.scalar.activation(std[t][:, :], MV[:, 2 * t + 1:2 * t + 2],
                                         Sqrt, bias=epsT[:, :])
                    nc.vector.reciprocal(RSD[:, t:t + 1], std[t][:, :])
                    nc.vector.tensor_scalar(
                        out=RSD[:, 8 + t:9 + t], in0=MV[:, 2 * t:2 * t + 1],
                        scalar1=RSD[:, t:t + 1], scalar2=-1.0,
                        op0=MULT, op1=MULT)
                    nc.scalar.activation(HN[t][:, :], opsT[t][:, :], Ident,
                                         scale=RSD[:, t:t + 1],
                                         bias=RSD[:, 8 + t:9 + t])

                def ftp_t(t, tag):
                    ftp = ps.tile([128, 512], BF16, tag=tag, name=f"ftp{t}_r{rp}")
                    for cc in range(4):
                        nc.tensor.transpose(
                            ftp[:, cc * 128:(cc + 1) * 128],
                            HN[t][:, cc * 128:(cc + 1) * 128], ID[:, :])
                    for cc in range(4):
                        dst = OUTS[:, t * 512 + cc * 128:t * 512 + (cc + 1) * 128]
                        srcv = ftp[:, cc * 128:(cc + 1) * 128]
                        if cc == 3:
                            nc.scalar.activation(dst, srcv, Ident,
                                                 scale=GM[:, cc:cc + 1],
                                                 bias=BT[:, cc:cc + 1])
                        else:
                            nc.vector.tensor_scalar(
                                out=dst, in0=srcv,
                                scalar1=GM[:, cc:cc + 1], scalar2=BT[:, cc:cc + 1],
                                op0=MULT, op1=ADD)

                ln_t(0); ln_t(1); ln_t(2); ln_t(3)
                ftp_t(0, "SA"); ftp_t(1, "SA")
                ftp_t(2, "SB"); ftp_t(3, "SB")
                nc.sync.dma_start(out=outp[:, 0:2048], in_=OUTS[:, 0:2048])
                for t in range(4, 8):
                    ln_t(t)
                    ftp_t(t, ("OA1", "OA2", "OB1", "OB2")[t - 4])
                    if t % 2 == 1:
                        nc.sync.dma_start(
                            out=outp[:, (t - 1) * 512:(t + 1) * 512],
                            in_=OUTS[:, (t - 1) * 512:(t + 1) * 512])

            for rp in range(reps):
                _body(rp)

    if split_waits:
        _split_mm_waits(nc)
    return nc


def _split_mm_waits(nc):
    """Walrus MM structs carry only one sync wait; move extras to a NoOp."""
    f = nc.m.functions[0]
    for bb in f.blocks:
        il = bb.instructions
        out, changed = [], False
        for i in il:
            si = getattr(i, "sync_info", None)
            tn = type(i).__name__
            splittable = tn.startswith("Inst") and tn not in ("InstNoOp", "InstAllEngineBarrier")
            if (splittable and si is not None
                    and si.on_wait is not None and len(si.on_wait) > 1):
                waits = list(si.on_wait)
                for wi, w in enumerate(waits[:-1]):
                    out.append(mybir.InstNoOp(
                        name=f"{i.name}-wsplit{wi}", engine=i.engine,
                        sync_info=mybir.SyncInfo(on_wait=[w], on_update=[])))
                i.sync_info = mybir.SyncInfo(
                    on_wait=[waits[-1]], on_update=list(si.on_update))
                changed = True
            out.append(i)
        if changed:
            bb.instructions = out


def _prep_inputs(x, sqi, w_qkv, w_out, b_out, w_conv, b_conv, ln_gamma, ln_beta):
    x = np.asarray(x, np.float32)
    sqi = np.asarray(sqi, np.float32)
    w_qkv = np.asarray(w_qkv, np.float32)
    w_out = np.asarray(w_out, np.float32)
    b_out = np.asarray(b_out, np.float32)
    w_conv = np.asarray(w_conv, np.float32)
    b_conv = np.asarray(b_conv, np.float32)
    ln_gamma = np.asarray(ln_gamma, np.float32)
    ln_beta = np.asarray(ln_beta, np.float32)

    sp = np.pad(sqi, ((0, 0), (1, 1)))
    bias = (w_conv[0] * sp[:, :-2] + w_conv[1] * sp[:, 1:-1]
            + w_conv[2] * sp[:, 2:] + b_conv)                    # (B, T)
    c = np.exp(bias).astype(np.float32)

    def wchunks(wT):
        return np.ascontiguousarray(
            wT.reshape(4, 128, C).transpose(1, 0, 2).reshape(128, 4 * C)
        ).astype(bf16)

    wqT = (w_qkv[:C].T * SCALE).astype(np.float32)
    wkT = w_qkv[C:2 * C].T.astype(np.float32)
    wvT = w_qkv[2 * C:].T.astype(np.float32)
    woT = w_out.T.astype(np.float32)
    wqm, wkm, wvm, wom = (wchunks(w) for w in (wqT, wkT, wvT, woT))
    gm = ln_gamma.reshape(4, 128).T.copy().astype(np.float32)
    bt = ln_beta.reshape(4, 128).T.copy().astype(np.float32)
    iden = np.eye(128, dtype=bf16)
    idenr = np.eye(128, dtype=np.float32)

    in_maps = []
    for core in range(8):
        b, qh = divmod(core, 2)
        qs = slice(qh * TQ, (qh + 1) * TQ)
        cb = c[b]
        cful = cb.reshape(KC, 128).T.copy().astype(np.float32)
        c8 = np.repeat(cb.reshape(KC, 128).T, H, axis=1).copy().astype(bf16)
        xb = x[b].astype(bf16)
        xctm = np.ascontiguousarray(
            xb.reshape(4, 128, T).transpose(1, 0, 2).reshape(128, 4 * T))
        xqm = np.ascontiguousarray(
            xb[:, qs].reshape(4, 128, TQ).transpose(1, 0, 2).reshape(128, 4 * TQ))
        xs = (x[b].T[qs] + b_out).astype(np.float32)            # (TQ, C)
        xsm = np.ascontiguousarray(
            xs.reshape(8, 128, C).transpose(1, 0, 2).reshape(128, 8 * C))
        in_maps.append({
            "xctm": xctm, "xqm": xqm, "xsm": xsm,
            "wqm": wqm, "wkm": wkm, "wvm": wvm, "wom": wom,
            "cful": cful, "c8": c8, "gmm": gm, "bet": bt,
            "iden": iden, "idenr": idenr,
        })
    return in_maps


def _unpack_out(o):
    """[128, t*512 + cc*128 + j] bf16 -> (C, TQ) f32."""
    o = np.asarray(o, dtype=np.float32).reshape(128, 8, 4, 128)
    return np.ascontiguousarray(o.transpose(2, 0, 1, 3)).reshape(C, TQ)


def kernel(x, sqi, w_qkv, w_out, b_out, w_conv, b_conv, ln_gamma, ln_beta,
           _trace=False):
    if "nc" not in _CACHE:
        _CACHE["nc"] = _build_nc()
    nc = _CACHE["nc"]
    in_maps = _prep_inputs(x, sqi, w_qkv, w_out, b_out, w_conv, b_conv,
                           ln_gamma, ln_beta)
    res = run_bass_kernel_spmd(nc, in_maps, core_ids=list(range(8)), trace=_trace)
    _CACHE["last_result"] = res
    out = np.empty((B, C, T), np.float32)
    for core in range(8):
        b, qh = divmod(core, 2)
        out[b][:, qh * TQ:(qh + 1) * TQ] = _unpack_out(res.results[core]["out"])
    return out


# revision 25
# speedup vs baseline: 1.0104x; 1.0104x over previous
"""Trainium2 Bass kernel for LogitBiasedSelfAttention1D.

Sharding: 8 cores = (batch b in 0..3) x (query half qh in 0..1).
Each core computes full attention (all 8 heads, all 2048 keys) for its
1024 queries of its batch. No collectives.

Math decomposition (exactly equivalent to the reference up to fp):
  - conv1d key bias folded into V:  softmax(S + bias) @ V
      = (exp(S) @ (c * V)) / (exp(S) @ c),   c = exp(bias)
  - SCALE folded into w_q on host.
  - residual x_seq + b_out added via identity matmul (f32r) into the
    out_proj PSUM accumulation.
  - LayerNorm normalize runs on ScalarE (Identity with scale/bias APs).

Schedule: the exp softmax on ScalarE is the bottleneck (~141us busy), so
everything is pipelined around keeping it fed:
  - QKV projections are emitted as small psum-chunk units interleaved into
    head-pair 0's S-loop (separate PSUM tags), so exp starts ~6us in.
  - PV (attn @ V) for pair p runs interleaved inside pair p+1's S-loop
    (PSUM accumulator banks conflict with projection psum at the head, so
    PV lags one pair; exp outputs buffer in a deep pt pool).
  - The LN tail is per-t-chunk pipelined across ACT/PE/DVE with the sqrt
    table preloaded during pair-3 PV, and the output DMA split in quarters.
"""

import sys

for _p in ("/opt/trn_rl_repo", "/root/.axon_site/_ro/trn_rl_repo"):
    if _p not in sys.path:
        sys.path.insert(0, _p)

import numpy as np
import ml_dtypes

from concourse import bass, mybir
from concourse.tile import TileContext
from concourse.bass_utils import run_bass_kernel_spmd

B, C, T = 4, 512, 2048
H, D = 8, 64
SCALE = D ** -0.5
EPS = 1e-5
TQ = T // 2            # queries per core
KC = T // 128          # 16 key chunks
PAIRS = H // 2         # 4 head pairs
F32 = mybir.dt.float32
F32R = mybir.dt.float32r
BF16 = mybir.dt.bfloat16
bf16 = ml_dtypes.bfloat16

Exp = mybir.ActivationFunctionType.Exp
SCH_A = float(2**7 / np.log(2))    # Schraudolph fast-exp, bf16 bits in int16
SCH_B = float(127 * 2**7 - 7.5)
Sqrt = mybir.ActivationFunctionType.Sqrt
Ident = mybir.ActivationFunctionType.Identity
MULT = mybir.AluOpType.mult
ADD = mybir.AluOpType.add

_CACHE = {}


def _build_nc(reps=1, split_waits=True):
    nc = bass.Bass()
    xctm = nc.declare_dram_parameter("xctm", [128, 4 * T], BF16, False)
    xqm = nc.declare_dram_parameter("xqm", [128, 4 * TQ], BF16, False)
    xsm = nc.declare_dram_parameter("xsm", [128, 8 * C], F32R, False)
    wqm = nc.declare_dram_parameter("wqm", [128, 4 * C], BF16, False)
    wkm = nc.declare_dram_parameter("wkm", [128, 4 * C], BF16, False)
    wvm = nc.declare_dram_parameter("wvm", [128, 4 * C], BF16, False)
    wom = nc.declare_dram_parameter("wom", [128, 4 * C], BF16, False)
    cful = nc.declare_dram_parameter("cful", [128, KC], F32, False)
    c8 = nc.declare_dram_parameter("c8", [128, KC * H], BF16, False)
    gmm = nc.declare_dram_parameter("gmm", [128, 4], F32, False)
    bet = nc.declare_dram_parameter("bet", [128, 4], F32, False)
    iden = nc.declare_dram_parameter("iden", [128, 128], BF16, False)
    idenr = nc.declare_dram_parameter("idenr", [128, 128], F32R, False)
    outp = nc.declare_dram_parameter("out", [128, 4 * TQ], BF16, True)

    with TileContext(nc) as tc:
        with (
            tc.sbuf_pool(name="cst", bufs=1) as cst,
            tc.sbuf_pool(name="pex", bufs=20) as pex,
            tc.sbuf_pool(name="sml", bufs=2) as sml,
            tc.psum_pool(name="ps", bufs=1) as ps,
        ):
            # ---- persistent state tiles ----
            epsT = cst.tile([128, 1], F32, name="epsT")
            dummy = cst.tile([128, 1], F32, name="dummy")
            KT = [cst.tile([128, T], BF16, name=f"KT{m}") for m in range(4)]
            QT = [cst.tile([128, TQ], BF16, name=f"QT{m}") for m in range(4)]
            VB = [cst.tile([128, H * 65], BF16, name=f"VB{k}") for k in range(KC)]
            OT = [cst.tile([128, TQ], BF16, name=f"OTp{p}") for p in range(PAIRS)]
            OUTS = cst.tile([128, 4 * TQ], BF16, name="OUTS")
            MV = cst.tile([128, 16], F32, name="MV")       # (mean, var) x 8 t
            RSD = cst.tile([128, 16], F32, name="RSD")     # rstd8 | -mu*rstd

            # preload the exp table while input DMAs run
            nc.vector.memset(epsT[:, :], EPS)
            nc.scalar.activation(dummy[:, :], epsT[:, :], Exp)

            # ---- input loads, compute-critical first ----
            WQb = cst.tile_from(wqm[:, :], name="WQb")
            XQb = cst.tile_from(xqm[:, :], name="XQb",
                                forced_dma_engine=mybir.EngineType.Pool)
            WKb = cst.tile_from(wkm[:, :], name="WKb")
            XCTb = cst.tile_from(xctm[:, :], name="XCTb",
                                 forced_dma_engine=mybir.EngineType.Pool)
            WVb = cst.tile_from(wvm[:, :], name="WVb")
            CF = cst.tile_from(cful[:, :], name="CF")
            C8 = cst.tile_from(c8[:, :], name="C8")
            ID = cst.tile_from(iden[:, :], name="ID")
            WOb = cst.tile_from(wom[:, :], name="WOb")
            IDR = cst.tile_from(idenr[:, :], name="IDR")
            XSb = cst.tile_from(xsm[:, :], name="XSb",
                                forced_dma_engine=mybir.EngineType.Pool)
            GM = cst.tile_from(gmm[:, :], name="GM")
            BT = cst.tile_from(bet[:, :], name="BT")
            WQ = [WQb[:, i * C:(i + 1) * C] for i in range(4)]
            XQ = [XQb[:, i * TQ:(i + 1) * TQ] for i in range(4)]
            WK = [WKb[:, i * C:(i + 1) * C] for i in range(4)]
            XCT = [XCTb[:, i * T:(i + 1) * T] for i in range(4)]
            WV = [WVb[:, i * C:(i + 1) * C] for i in range(4)]
            WO = [WOb[:, i * C:(i + 1) * C] for i in range(4)]
            XS = [XSb[:, t * C:(t + 1) * C] for t in range(8)]

            def _body(rp):
                # ---------- projection units (psum chunk each) ----------
                pcnt = [0]

                def q_unit(m, n):
                    qps = ps.tile([128, 512], F32,
                                  tag=("OA1", "OA2")[pcnt[0] % 2],
                                  name=f"qps{m}_{n}_r{rp}")
                    pcnt[0] += 1
                    for ci in range(4):
                        nc.tensor.matmul(
                            qps[:, :],
                            lhsT=WQ[ci][:, m * 128:(m + 1) * 128],
                            rhs=XQ[ci][:, n * 512:(n + 1) * 512],
                            start=(ci == 0), stop=(ci == 3))
                    nc.vector.tensor_copy(
                        QT[m][:, n * 512:(n + 1) * 512], qps[:, :])

                def k_unit(m, h2, n):
                    kps = ps.tile([128, 512], F32,
                                  tag=("OA1", "OA2")[pcnt[0] % 2],
                                  name=f"kps{m}_{h2}_{n}_r{rp}")
                    pcnt[0] += 1
                    for ci in range(4):
                        nc.tensor.matmul(
                            kps[:, :],
                            lhsT=WK[ci][:, m * 128:(m + 1) * 128],
                            rhs=XCT[ci][:, h2 * 1024 + n * 512:
                                        h2 * 1024 + (n + 1) * 512],
                            start=(ci == 0), stop=(ci == 3))
                    nc.vector.tensor_copy(
                        KT[m][:, h2 * 1024 + n * 512:h2 * 1024 + (n + 1) * 512],
                        kps[:, :])

                def v_unit(k):
                    vps = ps.tile([128, 512], F32, tag=("OB1", "OB2")[k % 2],
                                  name=f"vps{k}_r{rp}")
                    for ci in range(4):
                        nc.tensor.matmul(
                            vps[:, :],
                            lhsT=XCT[ci][:, k * 128:(k + 1) * 128],
                            rhs=WV[ci][:, :],
                            start=(ci == 0), stop=(ci == 3))
                    nc.vector.tensor_scalar(
                        out=VB[k].rearrange("p (h e) -> p h e", e=65)[:, :, 0:64],
                        in0=vps.rearrange("p (h e) -> p h e", e=64),
                        scalar1=CF[:, k:k + 1], scalar2=None, op0=MULT)
                    nc.vector.tensor_copy(
                        VB[k].rearrange("p (h e) -> p h e", e=65)[:, :, 64:65],
                        C8[:, k * H:(k + 1) * H].rearrange("p (h e) -> p h e", e=1))

                units = []
                units += [lambda n=n: k_unit(0, 1, n) for n in range(2)]
                units += [lambda n=n: q_unit(1, n) for n in range(2)]
                units += [lambda a=a: k_unit(1, *a)
                          for a in ((0, 0), (0, 1), (1, 0), (1, 1))]
                units += [lambda k=k: v_unit(k) for k in range(KC)]
                units += [lambda n=n: q_unit(2, n) for n in range(2)]
                units += [lambda a=a: k_unit(2, *a)
                          for a in ((0, 0), (0, 1), (1, 0), (1, 1))]
                units += [lambda n=n: q_unit(3, n) for n in range(2)]
                units += [lambda a=a: k_unit(3, *a)
                          for a in ((0, 0), (0, 1), (1, 0), (1, 1))]
                uidx = [0]

                def emit_units(cnt):
                    for _ in range(cnt):
                        if uidx[0] < len(units):
                            units[uidx[0]]()
                            uidx[0] += 1

                # upfront: QT0 fully, KT0 first half (covers S k<8)
                q_unit(0, 0); q_unit(0, 1); k_unit(0, 0, 0); k_unit(0, 0, 1)

                PT = {}    # (p, k, hi) -> pt tile, buffered until PV consumes
                last_pt = [None]

                def s_exp(p, k, hi, on_dve=False):
                    stag, ptag = ("SA", "pA") if hi == 0 else ("SB", "pB")
                    rows = slice(hi * 64, (hi + 1) * 64)
                    s_ps = ps.tile([128, 1024], F32, tag=stag,
                                   name=f"s{p}_{hi}_{k}_r{rp}")
                    for n in range(2):
                        nc.tensor.matmul(
                            s_ps[:, n * 512:(n + 1) * 512],
                            lhsT=KT[p][rows, k * 128:(k + 1) * 128],
                            rhs=QT[p][rows, n * 512:(n + 1) * 512],
                            start=True, stop=True)
                    pt = pex.tile([128, 1024], BF16, tag=ptag,
                                  name=f"pt{p}_{hi}_{k}_r{rp}")
                    if on_dve:
                        nc.vector.tensor_scalar(
                            out=pt.bitcast(mybir.dt.int16)[:, :], in0=s_ps[:, :],
                            scalar1=SCH_A, scalar2=SCH_B, op0=MULT, op1=ADD)
                    else:
                        nc.scalar.activation(pt[:, :], s_ps[:, :], Exp)
                        last_pt[0] = pt
                    PT[(p, k, hi)] = pt

                OACC = {}  # p -> {hi: [2 psum tiles]}

                def pv_half(p, k, hi):
                    if p not in OACC:
                        OACC[p] = {
                            0: [ps.tile([128, 512], F32, tag=("OA1", "OA2")[bk],
                                        name=f"o{p}_0_{bk}_r{rp}")
                                for bk in range(2)],
                            1: [ps.tile([128, 512], F32, tag=("OB1", "OB2")[bk],
                                        name=f"o{p}_1_{bk}_r{rp}")
                                for bk in range(2)],
                        }
                    pt = PT.pop((p, k, hi))
                    head = 2 * p + hi
                    for s in range(8):
                        bk, j = s // 4, s % 4
                        nc.tensor.matmul(
                            OACC[p][hi][bk][:, j * 65:(j + 1) * 65],
                            lhsT=pt[:, s * 128:(s + 1) * 128],
                            rhs=VB[k][:, head * 65:(head + 1) * 65],
                            start=(k == 0), stop=(k == KC - 1),
                            skip_group_check=True)

                def pv_chunk(p, k):
                    pv_half(p, k, 0)
                    pv_half(p, k, 1)

                def epilogue(p, use_act=False):
                    oacc = OACC.pop(p)
                    nmul = 0
                    ONs = [sml.tile([128, 128], BF16, tag="on",
                                    name=f"on{p}_{s}_r{rp}", bufs=10)
                           for s in range(8)]
                    for hi in (0, 1):
                        for bk in range(2):
                            o_ps = oacc[hi][bk]
                            ov = o_ps[:, 0:260].rearrange("p (s e) -> p s e", e=65)
                            rd4 = sml.tile([128, 4], F32, tag="rd",
                                           name=f"rd{p}_{hi}_{bk}_r{rp}", bufs=4)
                            nc.vector.reciprocal(
                                rd4.rearrange("p (s e) -> p s e", e=1),
                                ov[:, :, 64:65])
                            for j in range(4):
                                s = bk * 4 + j
                                nmul += 1
                                if use_act and nmul > 10:
                                    nc.scalar.activation(
                                        ONs[s][:, hi * 64:(hi + 1) * 64],
                                        o_ps[:, j * 65:j * 65 + 64],
                                        Ident, scale=rd4[:, j:j + 1])
                                else:
                                    nc.vector.tensor_scalar_mul(
                                        ONs[s][:, hi * 64:(hi + 1) * 64],
                                        o_ps[:, j * 65:j * 65 + 64],
                                        rd4[:, j:j + 1])
                    tps = [ps.tile([128, 512], BF16, tag=("OA1", "OA2")[b_],
                                   name=f"tp{p}_{b_}_r{rp}")
                           for b_ in range(2)]
                    for s in range(8):
                        nc.tensor.transpose(
                            tps[s // 4][:, (s % 4) * 128:(s % 4 + 1) * 128],
                            ONs[s][:, :], ID[:, :])
                    for b_ in range(2):
                        nc.vector.tensor_copy(OT[p][:, b_ * 512:(b_ + 1) * 512],
                                              tps[b_][:, :])

                # ---------- pair loops: S/exp for p, PV for p-1 ----------
                for p in range(PAIRS):
                    for k in range(KC):
                        s_exp(p, k, 0, on_dve=(p > 0 and k % 2 == 1))
                        if p > 0 and k > 0:
                            pv_half(p - 1, k - 1, 0)
                        s_exp(p, k, 1, on_dve=(p > 0 and k % 2 == 0))
                        if p == 0:
                            emit_units(3)
                        elif k > 0:
                            pv_half(p - 1, k - 1, 1)
                    emit_units(99)  # flush leftovers (end of pair 0 only)
                    if p > 0:
                        pv_chunk(p - 1, KC - 1)
                        epilogue(p - 1)

                # preload sqrt table right after the last exp (the read of
                # last_pt pins this behind the final Exp so the scheduler
                # cannot hoist it ahead and thrash the exp table)
                nc.scalar.activation(dummy[:, :], last_pt[0][:, 0:1], Sqrt)

                # ---------- PV(3) + early out_proj partials (t<4) ----------
                opsT = [None] * 8
                bigs = []
                for i in range(2):
                    big = ps.tile([128, 1024], F32, tag=("SA", "SB")[i],
                                  name=f"opb{i}_r{rp}")
                    bigs.append(big)
                    opsT[2 * i] = big[:, 0:512]
                    opsT[2 * i + 1] = big[:, 512:1024]
                for k in range(KC):
                    pv_chunk(3, k)
                    if k < 4:
                        t = k
                        for p_ in range(3):
                            nc.tensor.matmul(
                                opsT[t][:, :],
                                lhsT=OT[p_][:, t * 128:(t + 1) * 128],
                                rhs=WO[p_][:, :],
                                start=(p_ == 0), stop=False)
                epilogue(3, use_act=True)

                # ---------- finish out_proj + residual + LN stats ----------
                for t in range(8):
                    if t >= 4:
                        opsT[t] = ps.tile([128, 512], F32,
                                          tag=("OA1", "OA2", "OB1", "OB2")[t - 4],
                                          name=f"op{t}_r{rp}")
                        for p_ in range(4):
                            nc.tensor.matmul(
                                opsT[t][:, :],
                                lhsT=OT[p_][:, t * 128:(t + 1) * 128],
                                rhs=WO[p_][:, :],
                                start=(p_ == 0), stop=False)
                    else:
                        nc.tensor.matmul(
                            opsT[t][:, :],
                            lhsT=OT[3][:, t * 128:(t + 1) * 128],
                            rhs=WO[3][:, :],
                            start=False, stop=False)
                    nc.tensor.matmul(
                        opsT[t][:, :], lhsT=IDR[:, :], rhs=XS[t][:, :],
                        start=False, stop=True)
                    bnst = sml.tile([128, 6], F32, tag="bnst",
                                    name=f"bnst{t}_r{rp}", bufs=3)
                    nc.vector.bn_stats(bnst[:, :], opsT[t][:, :])
                    nc.vector.bn_aggr(MV[:, 2 * t:2 * t + 2], bnst[:, :])

                # ---------- per-t pipelined LN + transpose + store ----------
                HN = [sml.tile([128, C], BF16, tag="hn", name=f"hn{t}_r{rp}",
                               bufs=8) for t in range(8)]
                std = [sml.tile([128, 1], F32, tag="std", name=f"std{t}_r{rp}",
                                bufs=4) for t in range(8)]

                def ln_t(t):
                    nc.scalar.activation(std[t][:, :], MV[:, 2 * t + 1:2 * t + 2],
                                         Sqrt, bias=epsT[:, :])
                    nc.vector.reciprocal(RSD[:, t:t + 1], std[t][:, :])
                    nc.vector.tensor_scalar(
                        out=RSD[:, 8 + t:9 + t], in0=MV[:, 2 * t:2 * t + 1],
                        scalar1=RSD[:, t:t + 1], scalar2=-1.0,
                        op0=MULT, op1=MULT)
                    nc.scalar.activation(HN[t][:, :], opsT[t][:, :], Ident,
                                         scale=RSD[:, t:t + 1],
                                         bias=RSD[:, 8 + t:9 + t])

                def ftp_t(t, tag):
                    ftp = ps.tile([128, 512], BF16, tag=tag, name=f"ftp{t}_r{rp}")
                    for cc in range(4):
                        nc.tensor.transpose(
                            ftp[:, cc * 128:(cc + 1) * 128],
                            HN[t][:, cc * 128:(cc + 1) * 128], ID[:, :])
                    for cc in range(4):
                        dst = OUTS[:, t * 512 + cc * 128:t * 512 + (cc + 1) * 128]
                        srcv = ftp[:, cc * 128:(cc + 1) * 128]
                        if cc == 3:
                            nc.scalar.activation(dst, srcv, Ident,
                                                 scale=GM[:, cc:cc + 1],
                                                 bias=BT[:, cc:cc + 1])
                        else:
                            nc.vector.tensor_scalar(
                                out=dst, in0=srcv,
                                scalar1=GM[:, cc:cc + 1], scalar2=BT[:, cc:cc + 1],
                                op0=MULT, op1=ADD)

                ln_t(0); ln_t(1); ln_t(2); ln_t(3)
                ftp_t(0, "SA"); ftp_t(1, "SA")
                ftp_t(2, "SB"); ftp_t(3, "SB")
                nc.sync.dma_start(out=outp[:, 0:2048], in_=OUTS[:, 0:2048])
                for t in range(4, 8):
                    ln_t(t)
                    ftp_t(t, ("OA1", "OA2", "OB1", "OB2")[t - 4])
                    if t % 2 == 1:
                        nc.sync.dma_start(
                            out=outp[:, (t - 1) * 512:(t + 1) * 512],
                            in_=OUTS[:, (t - 1) * 512:(t + 1) * 512])

            for rp in range(reps):
                _body(rp)

    if split_waits:
        _split_mm_waits(nc)
    return nc


def _split_mm_waits(nc):
    """Walrus MM structs carry only one sync wait; move extras to a NoOp."""
    f = nc.m.functions[0]
    for bb in f.blocks:
        il = bb.instructions
        out, changed = [], False
        for i in il:
            si = getattr(i, "sync_info", None)
            tn = type(i).__name__
            splittable = tn.startswith("Inst") and tn not in ("InstNoOp", "InstAllEngineBarrier")
            if (splittable and si is not None
                    and si.on_wait is not None and len(si.on_wait) > 1):
                waits = list(si.on_wait)
                for wi, w in enumerate(waits[:-1]):
                    out.append(mybir.InstNoOp(
                        name=f"{i.name}-wsplit{wi}", engine=i.engine,
                        sync_info=mybir.SyncInfo(on_wait=[w], on_update=[])))
                i.sync_info = mybir.SyncInfo(
                    on_wait=[waits[-1]], on_update=list(si.on_update))
                changed = True
            out.append(i)
        if changed:
            bb.instructions = out


def _prep_inputs(x, sqi, w_qkv, w_out, b_out, w_conv, b_conv, ln_gamma, ln_beta):
    x = np.asarray(x, np.float32)
    sqi = np.asarray(sqi, np.float32)
    w_qkv = np.asarray(w_qkv, np.float32)
    w_out = np.asarray(w_out, np.float32)
    b_out = np.asarray(b_out, np.float32)
    w_conv = np.asarray(w_conv, np.float32)
    b_conv = np.asarray(b_conv, np.float32)
    ln_gamma = np.asarray(ln_gamma, np.float32)
    ln_beta = np.asarray(ln_beta, np.float32)

    sp = np.pad(sqi, ((0, 0), (1, 1)))
    bias = (w_conv[0] * sp[:, :-2] + w_conv[1] * sp[:, 1:-1]
            + w_conv[2] * sp[:, 2:] + b_conv)                    # (B, T)
    c = np.exp(bias).astype(np.float32)

    def wchunks(wT):
        return np.ascontiguousarray(
            wT.reshape(4, 128, C).transpose(1, 0, 2).reshape(128, 4 * C)
        ).astype(bf16)

    wqT = (w_qkv[:C].T * SCALE).astype(np.float32)
    wkT = w_qkv[C:2 * C].T.astype(np.float32)
    wvT = w_qkv[2 * C:].T.astype(np.float32)
    woT = w_out.T.astype(np.float32)
    wqm, wkm, wvm, wom = (wchunks(w) for w in (wqT, wkT, wvT, woT))
    gm = ln_gamma.reshape(4, 128).T.copy().astype(np.float32)
    bt = ln_beta.reshape(4, 128).T.copy().astype(np.float32)
    iden = np.eye(128, dtype=bf16)
    idenr = np.eye(128, dtype=np.float32)

    in_maps = []
    for core in range(8):
        b, qh = divmod(core, 2)
        qs = slice(qh * TQ, (qh + 1) * TQ)
        cb = c[b]
        cful = cb.reshape(KC, 128).T.copy().astype(np.float32)
        c8 = np.repeat(cb.reshape(KC, 128).T, H, axis=1).copy().astype(bf16)
        xb = x[b].astype(bf16)
        xctm = np.ascontiguousarray(
            xb.reshape(4, 128, T).transpose(1, 0, 2).reshape(128, 4 * T))
        xqm = np.ascontiguousarray(
            xb[:, qs].reshape(4, 128, TQ).transpose(1, 0, 2).reshape(128, 4 * TQ))
        xs = (x[b].T[qs] + b_out).astype(np.float32)            # (TQ, C)
        xsm = np.ascontiguousarray(
            xs.reshape(8, 128, C).transpose(1, 0, 2).reshape(128, 8 * C))
        in_maps.append({
            "xctm": xctm, "xqm": xqm, "xsm": xsm,
            "wqm": wqm, "wkm": wkm, "wvm": wvm, "wom": wom,
            "cful": cful, "c8": c8, "gmm": gm, "bet": bt,
            "iden": iden, "idenr": idenr,
        })
    return in_maps


def _unpack_out(o):
    """[128, t*512 + cc*128 + j] bf16 -> (C, TQ) f32."""
    o = np.asarray(o, dtype=np.float32).reshape(128, 8, 4, 128)
    return np.ascontiguousarray(o.transpose(2, 0, 1, 3)).reshape(C, TQ)


def kernel(x, sqi, w_qkv, w_out, b_out, w_conv, b_conv, ln_gamma, ln_beta,
           _trace=False):
    if "nc" not in _CACHE:
        _CACHE["nc"] = _build_nc()
    nc = _CACHE["nc"]
    in_maps = _prep_inputs(x, sqi, w_qkv, w_out, b_out, w_conv, b_conv,
                           ln_gamma, ln_beta)
    res = run_bass_kernel_spmd(nc, in_maps, core_ids=list(range(8)), trace=_trace)
    _CACHE["last_result"] = res
    out = np.empty((B, C, T), np.float32)
    for core in range(8):
        b, qh = divmod(core, 2)
        out[b][:, qh * TQ:(qh + 1) * TQ] = _unpack_out(res.results[core]["out"])
    return out


# revision 33
# speedup vs baseline: 1.0423x; 1.0316x over previous
"""Trainium2 Bass kernel for LogitBiasedSelfAttention1D.

Sharding: 8 cores = (batch b in 0..3) x (query half qh in 0..1).
Each core computes full attention (all 8 heads, all 2048 keys) for its
1024 queries of its batch. No collectives.

Math decomposition (exactly equivalent to the reference up to fp):
  - conv1d key bias folded into V:  softmax(S + bias) @ V
      = (exp(S) @ (c * V)) / (exp(S) @ c),   c = exp(bias)
  - SCALE folded into w_q on host.
  - residual x_seq + b_out added via identity matmul (f32r) into the
    out_proj PSUM accumulation.
  - LayerNorm normalize runs on ScalarE (Identity with scale/bias APs).

Schedule: the exp softmax on ScalarE is the bottleneck (~141us busy), so
everything is pipelined around keeping it fed:
  - QKV projections are emitted as small psum-chunk units interleaved into
    head-pair 0's S-loop (separate PSUM tags), so exp starts ~6us in.
  - PV (attn @ V) for pair p runs interleaved inside pair p+1's S-loop
    (PSUM accumulator banks conflict with projection psum at the head, so
    PV lags one pair; exp outputs buffer in a deep pt pool).
  - The LN tail is per-t-chunk pipelined across ACT/PE/DVE with the sqrt
    table preloaded during pair-3 PV, and the output DMA split in quarters.
"""

import sys

for _p in ("/opt/trn_rl_repo", "/root/.axon_site/_ro/trn_rl_repo"):
    if _p not in sys.path:
        sys.path.insert(0, _p)

import numpy as np
import ml_dtypes

from concourse import bass, mybir
from concourse.tile import TileContext
from concourse.bass_utils import run_bass_kernel_spmd

B, C, T = 4, 512, 2048
H, D = 8, 64
SCALE = D ** -0.5
EPS = 1e-5
TQ = T // 2            # queries per core
KC = T // 128          # 16 key chunks
PAIRS = H // 2         # 4 head pairs
F32 = mybir.dt.float32
F32R = mybir.dt.float32r
BF16 = mybir.dt.bfloat16
bf16 = ml_dtypes.bfloat16

Exp = mybir.ActivationFunctionType.Exp
SCH_A = float(2**7 / np.log(2))    # Schraudolph fast-exp, bf16 bits in int16
SCH_B = float(127 * 2**7 - 7.5)
Sqrt = mybir.ActivationFunctionType.Sqrt
Ident = mybir.ActivationFunctionType.Identity
MULT = mybir.AluOpType.mult
ADD = mybir.AluOpType.add

_CACHE = {}


def _build_nc(reps=1, split_waits=True):
    nc = bass.Bass()
    xctm = nc.declare_dram_parameter("xctm", [128, 4 * T], BF16, False)
    xqm = nc.declare_dram_parameter("xqm", [128, 4 * TQ], BF16, False)
    xsm = nc.declare_dram_parameter("xsm", [128, 8 * C], F32R, False)
    wqm = nc.declare_dram_parameter("wqm", [128, 4 * C], BF16, False)
    wkm = nc.declare_dram_parameter("wkm", [128, 4 * C], BF16, False)
    wvm = nc.declare_dram_parameter("wvm", [128, 4 * C], BF16, False)
    wom = nc.declare_dram_parameter("wom", [128, 4 * C], BF16, False)
    cful = nc.declare_dram_parameter("cful", [128, KC], F32, False)
    c8 = nc.declare_dram_parameter("c8", [128, KC * H], BF16, False)
    gmm = nc.declare_dram_parameter("gmm", [128, 4], F32, False)
    bet = nc.declare_dram_parameter("bet", [128, 4], F32, False)
    iden = nc.declare_dram_parameter("iden", [128, 128], BF16, False)
    idenr = nc.declare_dram_parameter("idenr", [128, 128], F32R, False)
    outp = nc.declare_dram_parameter("out", [128, 4 * TQ], BF16, True)

    with TileContext(nc) as tc:
        with (
            tc.sbuf_pool(name="cst", bufs=1) as cst,
            tc.sbuf_pool(name="pex", bufs=20) as pex,
            tc.sbuf_pool(name="sml", bufs=2) as sml,
            tc.psum_pool(name="ps", bufs=1) as ps,
        ):
            # ---- persistent state tiles ----
            epsT = cst.tile([128, 1], F32, name="epsT")
            dummy = cst.tile([128, 1], F32, name="dummy")
            KT = [cst.tile([128, T], BF16, name=f"KT{m}") for m in range(4)]
            QT = [cst.tile([128, TQ], BF16, name=f"QT{m}") for m in range(4)]
            VB = [cst.tile([128, H * 65], BF16, name=f"VB{k}") for k in range(KC)]
            OT = [cst.tile([128, TQ], BF16, name=f"OTp{p}") for p in range(PAIRS)]
            OUTS = cst.tile([128, 4 * TQ], BF16, name="OUTS")
            MV = cst.tile([128, 16], F32, name="MV")       # (mean, var) x 8 t
            RSD = cst.tile([128, 16], F32, name="RSD")     # rstd8 | -mu*rstd

            # preload the exp table while input DMAs run
            nc.vector.memset(epsT[:, :], EPS)
            nc.scalar.activation(dummy[:, :], epsT[:, :], Exp)

            # ---- input loads, compute-critical first ----
            WQb = cst.tile_from(wqm[:, :], name="WQb")
            XQb = cst.tile_from(xqm[:, :], name="XQb",
                                forced_dma_engine=mybir.EngineType.Pool)
            WKb = cst.tile_from(wkm[:, :], name="WKb")
            XCTb = cst.tile_from(xctm[:, :], name="XCTb",
                                 forced_dma_engine=mybir.EngineType.Pool)
            WVb = cst.tile_from(wvm[:, :], name="WVb")
            CF = cst.tile_from(cful[:, :], name="CF")
            C8 = cst.tile_from(c8[:, :], name="C8")
            ID = cst.tile_from(iden[:, :], name="ID")
            WOb = cst.tile_from(wom[:, :], name="WOb")
            IDR = cst.tile_from(idenr[:, :], name="IDR")
            XSb = cst.tile_from(xsm[:, :], name="XSb",
                                forced_dma_engine=mybir.EngineType.Pool)
            GM = cst.tile_from(gmm[:, :], name="GM")
            BT = cst.tile_from(bet[:, :], name="BT")
            WQ = [WQb[:, i * C:(i + 1) * C] for i in range(4)]
            XQ = [XQb[:, i * TQ:(i + 1) * TQ] for i in range(4)]
            WK = [WKb[:, i * C:(i + 1) * C] for i in range(4)]
            XCT = [XCTb[:, i * T:(i + 1) * T] for i in range(4)]
            WV = [WVb[:, i * C:(i + 1) * C] for i in range(4)]
            WO = [WOb[:, i * C:(i + 1) * C] for i in range(4)]
            XS = [XSb[:, t * C:(t + 1) * C] for t in range(8)]

            def _body(rp):
                # ---------- projection units (psum chunk each) ----------
                pcnt = [0]

                def q_unit(m, n):
                    qps = ps.tile([128, 512], F32,
                                  tag=("OA1", "OA2")[pcnt[0] % 2],
                                  name=f"qps{m}_{n}_r{rp}")
                    pcnt[0] += 1
                    for ci in range(4):
                        nc.tensor.matmul(
                            qps[:, :],
                            lhsT=WQ[ci][:, m * 128:(m + 1) * 128],
                            rhs=XQ[ci][:, n * 512:(n + 1) * 512],
                            start=(ci == 0), stop=(ci == 3))
                    nc.vector.tensor_copy(
                        QT[m][:, n * 512:(n + 1) * 512], qps[:, :])

                def k_unit(m, h2, n):
                    kps = ps.tile([128, 512], F32,
                                  tag=("OA1", "OA2")[pcnt[0] % 2],
                                  name=f"kps{m}_{h2}_{n}_r{rp}")
                    pcnt[0] += 1
                    for ci in range(4):
                        nc.tensor.matmul(
                            kps[:, :],
                            lhsT=WK[ci][:, m * 128:(m + 1) * 128],
                            rhs=XCT[ci][:, h2 * 1024 + n * 512:
                                        h2 * 1024 + (n + 1) * 512],
                            start=(ci == 0), stop=(ci == 3))
                    nc.vector.tensor_copy(
                        KT[m][:, h2 * 1024 + n * 512:h2 * 1024 + (n + 1) * 512],
                        kps[:, :])

                def v_unit(k):
                    vps = ps.tile([128, 512], F32, tag=("OB1", "OB2")[k % 2],
                                  name=f"vps{k}_r{rp}")
                    for ci in range(4):
                        nc.tensor.matmul(
                            vps[:, :],
                            lhsT=XCT[ci][:, k * 128:(k + 1) * 128],
                            rhs=WV[ci][:, :],
                            start=(ci == 0), stop=(ci == 3))
                    nc.vector.tensor_scalar(
                        out=VB[k].rearrange("p (h e) -> p h e", e=65)[:, :, 0:64],
                        in0=vps.rearrange("p (h e) -> p h e", e=64),
                        scalar1=CF[:, k:k + 1], scalar2=None, op0=MULT)
                    nc.vector.tensor_copy(
                        VB[k].rearrange("p (h e) -> p h e", e=65)[:, :, 64:65],
                        C8[:, k * H:(k + 1) * H].rearrange("p (h e) -> p h e", e=1))

                units = []
                units += [lambda n=n: k_unit(0, 1, n) for n in range(2)]
                units += [lambda n=n: q_unit(1, n) for n in range(2)]
                units += [lambda a=a: k_unit(1, *a)
                          for a in ((0, 0), (0, 1), (1, 0), (1, 1))]
                units += [lambda k=k: v_unit(k) for k in range(KC)]
                units += [lambda n=n: q_unit(2, n) for n in range(2)]
                units += [lambda a=a: k_unit(2, *a)
                          for a in ((0, 0), (0, 1), (1, 0), (1, 1))]
                units += [lambda n=n: q_unit(3, n) for n in range(2)]
                units += [lambda a=a: k_unit(3, *a)
                          for a in ((0, 0), (0, 1), (1, 0), (1, 1))]
                uidx = [0]

                def emit_units(cnt):
                    for _ in range(cnt):
                        if uidx[0] < len(units):
                            units[uidx[0]]()
                            uidx[0] += 1

                # upfront: QT0 fully, KT0 first half (covers S k<8)
                q_unit(0, 0); q_unit(0, 1); k_unit(0, 0, 0); k_unit(0, 0, 1)

                PT = {}    # (p, k, hi) -> pt tile, buffered until PV consumes
                last_pt = [None]

                def s_exp(p, k, hi, on_dve=False):
                    stag, ptag = ("SA", "pA") if hi == 0 else ("SB", "pB")
                    rows = slice(hi * 64, (hi + 1) * 64)
                    s_ps = ps.tile([128, 1024], F32, tag=stag,
                                   name=f"s{p}_{hi}_{k}_r{rp}")
                    for n in range(2):
                        nc.tensor.matmul(
                            s_ps[:, n * 512:(n + 1) * 512],
                            lhsT=KT[p][rows, k * 128:(k + 1) * 128],
                            rhs=QT[p][rows, n * 512:(n + 1) * 512],
                            start=True, stop=True)
                    pt = pex.tile([128, 1024], BF16, tag=ptag,
                                  name=f"pt{p}_{hi}_{k}_r{rp}")
                    if on_dve:
                        nc.vector.tensor_scalar(
                            out=pt.bitcast(mybir.dt.int16)[:, :], in0=s_ps[:, :],
                            scalar1=SCH_A, scalar2=SCH_B, op0=MULT, op1=ADD)
                    else:
                        nc.scalar.activation(pt[:, :], s_ps[:, :], Exp)
                        last_pt[0] = pt
                    PT[(p, k, hi)] = pt

                OACC = {}  # p -> {hi: [2 psum tiles]}

                def pv_half(p, k, hi):
                    if p not in OACC:
                        OACC[p] = {
                            0: [ps.tile([128, 512], F32, tag=("OA1", "OA2")[bk],
                                        name=f"o{p}_0_{bk}_r{rp}")
                                for bk in range(2)],
                            1: [ps.tile([128, 512], F32, tag=("OB1", "OB2")[bk],
                                        name=f"o{p}_1_{bk}_r{rp}")
                                for bk in range(2)],
                        }
                    pt = PT.pop((p, k, hi))
                    head = 2 * p + hi
                    for s in range(8):
                        bk, j = s // 4, s % 4
                        nc.tensor.matmul(
                            OACC[p][hi][bk][:, j * 65:(j + 1) * 65],
                            lhsT=pt[:, s * 128:(s + 1) * 128],
                            rhs=VB[k][:, head * 65:(head + 1) * 65],
                            start=(k == 0), stop=(k == KC - 1),
                            skip_group_check=True)

                def pv_chunk(p, k):
                    pv_half(p, k, 0)
                    pv_half(p, k, 1)

                def epilogue(p, use_act=False):
                    oacc = OACC.pop(p)
                    nmul = 0
                    ONs = [sml.tile([128, 128], BF16, tag="on",
                                    name=f"on{p}_{s}_r{rp}", bufs=10)
                           for s in range(8)]
                    for hi in (0, 1):
                        for bk in range(2):
                            o_ps = oacc[hi][bk]
                            ov = o_ps[:, 0:260].rearrange("p (s e) -> p s e", e=65)
                            rd4 = sml.tile([128, 4], F32, tag="rd",
                                           name=f"rd{p}_{hi}_{bk}_r{rp}", bufs=4)
                            nc.vector.reciprocal(
                                rd4.rearrange("p (s e) -> p s e", e=1),
                                ov[:, :, 64:65])
                            for j in range(4):
                                s = bk * 4 + j
                                nmul += 1
                                if use_act and nmul > 10:
                                    nc.scalar.activation(
                                        ONs[s][:, hi * 64:(hi + 1) * 64],
                                        o_ps[:, j * 65:j * 65 + 64],
                                        Ident, scale=rd4[:, j:j + 1])
                                else:
                                    nc.vector.tensor_scalar_mul(
                                        ONs[s][:, hi * 64:(hi + 1) * 64],
                                        o_ps[:, j * 65:j * 65 + 64],
                                        rd4[:, j:j + 1])
                    tps = [ps.tile([128, 512], BF16, tag=("OA1", "OA2")[b_],
                                   name=f"tp{p}_{b_}_r{rp}")
                           for b_ in range(2)]
                    for s in range(8):
                        nc.tensor.transpose(
                            tps[s // 4][:, (s % 4) * 128:(s % 4 + 1) * 128],
                            ONs[s][:, :], ID[:, :])
                    for b_ in range(2):
                        nc.vector.tensor_copy(OT[p][:, b_ * 512:(b_ + 1) * 512],
                                              tps[b_][:, :])

                # ---------- pair loops: S/exp for p, PV for p-1 ----------
                for p in range(PAIRS):
                    for k in range(KC):
                        s_exp(p, k, 0, on_dve=(p > 0 and k % 2 == 1))
                        if p > 0 and k > 0:
                            pv_half(p - 1, k - 1, 0)
                        s_exp(p, k, 1, on_dve=(p > 0 and k % 2 == 0))
                        if p == 0:
                            emit_units(1 + (k % 2))
                        elif k > 0:
                            pv_half(p - 1, k - 1, 1)
                    emit_units(99)  # flush leftovers (end of pair 0 only)
                    if p > 0:
                        pv_chunk(p - 1, KC - 1)
                        epilogue(p - 1)

                # preload sqrt table right after the last exp (the read of
                # last_pt pins this behind the final Exp so the scheduler
                # cannot hoist it ahead and thrash the exp table)
                nc.scalar.activation(dummy[:, :], last_pt[0][:, 0:1], Sqrt)

                # ---------- PV(3) + early out_proj partials (t<4) ----------
                opsT = [None] * 8
                bigs = []
                for i in range(2):
                    big = ps.tile([128, 1024], F32, tag=("SA", "SB")[i],
                                  name=f"opb{i}_r{rp}")
                    bigs.append(big)
                    opsT[2 * i] = big[:, 0:512]
                    opsT[2 * i + 1] = big[:, 512:1024]
                for k in range(KC):
                    pv_chunk(3, k)
                    if k < 4:
                        t = k
                        for p_ in range(3):
                            nc.tensor.matmul(
                                opsT[t][:, :],
                                lhsT=OT[p_][:, t * 128:(t + 1) * 128],
                                rhs=WO[p_][:, :],
                                start=(p_ == 0), stop=False)
                epilogue(3, use_act=True)

                # ---------- finish out_proj + residual + LN stats ----------
                for t in range(8):
                    if t >= 4:
                        opsT[t] = ps.tile([128, 512], F32,
                                          tag=("OA1", "OA2", "OB1", "OB2")[t - 4],
                                          name=f"op{t}_r{rp}")
                        for p_ in range(4):
                            nc.tensor.matmul(
                                opsT[t][:, :],
                                lhsT=OT[p_][:, t * 128:(t + 1) * 128],
                                rhs=WO[p_][:, :],
                                start=(p_ == 0), stop=False)
                    else:
                        nc.tensor.matmul(
                            opsT[t][:, :],
                            lhsT=OT[3][:, t * 128:(t + 1) * 128],
                            rhs=WO[3][:, :],
                            start=False, stop=False)
                    nc.tensor.matmul(
                        opsT[t][:, :], lhsT=IDR[:, :], rhs=XS[t][:, :],
                        start=False, stop=True)
                    bnst = sml.tile([128, 6], F32, tag="bnst",
                                    name=f"bnst{t}_r{rp}", bufs=3)
                    nc.vector.bn_stats(bnst[:, :], opsT[t][:, :])
                    nc.vector.bn_aggr(MV[:, 2 * t:2 * t + 2], bnst[:, :])

                # ---------- per-t pipelined LN + transpose + store ----------
                HN = [sml.tile([128, C], BF16, tag="hn", name=f"hn{t}_r{rp}",
                               bufs=8) for t in range(8)]
                std = [sml.tile([128, 1], F32, tag="std", name=f"std{t}_r{rp}",
                                bufs=4) for t in range(8)]

                def ln_t(t):
                    nc.scalar.activation(std[t][:, :], MV[:, 2 * t + 1:2 * t + 2],
                                         Sqrt, bias=epsT[:, :])
                    nc.vector.reciprocal(RSD[:, t:t + 1], std[t][:, :])
                    nc.vector.tensor_scalar(
                        out=RSD[:, 8 + t:9 + t], in0=MV[:, 2 * t:2 * t + 1],
                        scalar1=RSD[:, t:t + 1], scalar2=-1.0,
                        op0=MULT, op1=MULT)
                    nc.scalar.activation(HN[t][:, :], opsT[t][:, :], Ident,
                                         scale=RSD[:, t:t + 1],
                                         bias=RSD[:, 8 + t:9 + t])

                def ftp_t(t, tag):
                    ftp = ps.tile([128, 512], BF16, tag=tag, name=f"ftp{t}_r{rp}")
                    for cc in range(4):
                        nc.tensor.transpose(
                            ftp[:, cc * 128:(cc + 1) * 128],
                            HN[t][:, cc * 128:(cc + 1) * 128], ID[:, :])
                    for cc in range(4):
                        dst = OUTS[:, t * 512 + cc * 128:t * 512 + (cc + 1) * 128]
                        srcv = ftp[:, cc * 128:(cc + 1) * 128]
                        if cc == 3:
                            nc.scalar.activation(dst, srcv, Ident,
                                                 scale=GM[:, cc:cc + 1],
                                                 bias=BT[:, cc:cc + 1])
                        else:
                            nc.vector.tensor_scalar(
                                out=dst, in0=srcv,
                                scalar1=GM[:, cc:cc + 1], scalar2=BT[:, cc:cc + 1],
                                op0=MULT, op1=ADD)

                ln_t(0); ln_t(1); ln_t(2); ln_t(3)
                ftp_t(0, "SA"); ftp_t(1, "SA")
                ftp_t(2, "SB"); ftp_t(3, "SB")
                nc.sync.dma_start(out=outp[:, 0:2048], in_=OUTS[:, 0:2048])
                for t in range(4, 8):
                    ln_t(t)
                    ftp_t(t, ("OA1", "OA2", "OB1", "OB2")[t - 4])
                    if t % 2 == 1:
                        nc.sync.dma_start(
                            out=outp[:, (t - 1) * 512:(t + 1) * 512],
                            in_=OUTS[:, (t - 1) * 512:(t + 1) * 512])

            for rp in range(reps):
                _body(rp)

    if split_waits:
        _split_mm_waits(nc)
    return nc


def _split_mm_waits(nc):
    """Walrus MM structs carry only one sync wait; move extras to a NoOp."""
    f = nc.m.functions[0]
    for bb in f.blocks:
        il = bb.instructions
        out, changed = [], False
        for i in il:
            si = getattr(i, "sync_info", None)
            tn = type(i).__name__
            splittable = tn.startswith("Inst") and tn not in ("InstNoOp", "InstAllEngineBarrier")
            if (splittable and si is not None
                    and si.on_wait is not None and len(si.on_wait) > 1):
                waits = list(si.on_wait)
                for wi, w in enumerate(waits[:-1]):
                    out.append(mybir.InstNoOp(
                        name=f"{i.name}-wsplit{wi}", engine=i.engine,
                        sync_info=mybir.SyncInfo(on_wait=[w], on_update=[])))
                i.sync_info = mybir.SyncInfo(
                    on_wait=[waits[-1]], on_update=list(si.on_update))
                changed = True
            out.append(i)
        if changed:
            bb.instructions = out


def _prep_inputs(x, sqi, w_qkv, w_out, b_out, w_conv, b_conv, ln_gamma, ln_beta):
    x = np.asarray(x, np.float32)
    sqi = np.asarray(sqi, np.float32)
    w_qkv = np.asarray(w_qkv, np.float32)
    w_out = np.asarray(w_out, np.float32)
    b_out = np.asarray(b_out, np.float32)
    w_conv = np.asarray(w_conv, np.float32)
    b_conv = np.asarray(b_conv, np.float32)
    ln_gamma = np.asarray(ln_gamma, np.float32)
    ln_beta = np.asarray(ln_beta, np.float32)

    sp = np.pad(sqi, ((0, 0), (1, 1)))
    bias = (w_conv[0] * sp[:, :-2] + w_conv[1] * sp[:, 1:-1]
            + w_conv[2] * sp[:, 2:] + b_conv)                    # (B, T)
    c = np.exp(bias).astype(np.float32)

    def wchunks(wT):
        return np.ascontiguousarray(
            wT.reshape(4, 128, C).transpose(1, 0, 2).reshape(128, 4 * C)
        ).astype(bf16)

    wqT = (w_qkv[:C].T * SCALE).astype(np.float32)
    wkT = w_qkv[C:2 * C].T.astype(np.float32)
    wvT = w_qkv[2 * C:].T.astype(np.float32)
    woT = w_out.T.astype(np.float32)
    wqm, wkm, wvm, wom = (wchunks(w) for w in (wqT, wkT, wvT, woT))
    gm = ln_gamma.reshape(4, 128).T.copy().astype(np.float32)
    bt = ln_beta.reshape(4, 128).T.copy().astype(np.float32)
    iden = np.eye(128, dtype=bf16)
    idenr = np.eye(128, dtype=np.float32)

    in_maps = []
    for core in range(8):
        b, qh = divmod(core, 2)
        qs = slice(qh * TQ, (qh + 1) * TQ)
        cb = c[b]
        cful = cb.reshape(KC, 128).T.copy().astype(np.float32)
        c8 = np.repeat(cb.reshape(KC, 128).T, H, axis=1).copy().astype(bf16)
        xb = x[b].astype(bf16)
        xctm = np.ascontiguousarray(
            xb.reshape(4, 128, T).transpose(1, 0, 2).reshape(128, 4 * T))
        xqm = np.ascontiguousarray(
            xb[:, qs].reshape(4, 128, TQ).transpose(1, 0, 2).reshape(128, 4 * TQ))
        xs = (x[b].T[qs] + b_out).astype(np.float32)            # (TQ, C)
        xsm = np.ascontiguousarray(
            xs.reshape(8, 128, C).transpose(1, 0, 2).reshape(128, 8 * C))
        in_maps.append({
            "xctm": xctm, "xqm": xqm, "xsm": xsm,
            "wqm": wqm, "wkm": wkm, "wvm": wvm, "wom": wom,
            "cful": cful, "c8": c8, "gmm": gm, "bet": bt,
            "iden": iden, "idenr": idenr,
        })
    return in_maps


def _unpack_out(o):
    """[128, t*512 + cc*128 + j] bf16 -> (C, TQ) f32."""
    o = np.asarray(o, dtype=np.float32).reshape(128, 8, 4, 128)
    return np.ascontiguousarray(o.transpose(2, 0, 1, 3)).reshape(C, TQ)


def kernel(x, sqi, w_qkv, w_out, b_out, w_conv, b_conv, ln_gamma, ln_beta,
           _trace=False):
    if "nc" not in _CACHE:
        _CACHE["nc"] = _build_nc()
    nc = _CACHE["nc"]
    in_maps = _prep_inputs(x, sqi, w_qkv, w_out, b_out, w_conv, b_conv,
                           ln_gamma, ln_beta)
    res = run_bass_kernel_spmd(nc, in_maps, core_ids=list(range(8)), trace=_trace)
    _CACHE["last_result"] = res
    out = np.empty((B, C, T), np.float32)
    for core in range(8):
        b, qh = divmod(core, 2)
        out[b][:, qh * TQ:(qh + 1) * TQ] = _unpack_out(res.results[core]["out"])
    return out


# revision 41
# speedup vs baseline: 1.0629x; 1.0197x over previous
"""Trainium2 Bass kernel for LogitBiasedSelfAttention1D.

Sharding: 8 cores = (batch b in 0..3) x (query half qh in 0..1).
Each core computes full attention (all 8 heads, all 2048 keys) for its
1024 queries of its batch. No collectives.

Math decomposition (exactly equivalent to the reference up to fp):
  - conv1d key bias folded into V:  softmax(S + bias) @ V
      = (exp(S) @ (c * V)) / (exp(S) @ c),   c = exp(bias)
  - SCALE folded into w_q on host.
  - residual x_seq + b_out added via identity matmul (f32r) into the
    out_proj PSUM accumulation.
  - LayerNorm normalize runs on ScalarE (Identity with scale/bias APs).

Schedule: the exp softmax on ScalarE is the bottleneck (~141us busy), so
everything is pipelined around keeping it fed:
  - QKV projections are emitted as small psum-chunk units interleaved into
    head-pair 0's S-loop (separate PSUM tags), so exp starts ~6us in.
  - PV (attn @ V) for pair p runs interleaved inside pair p+1's S-loop
    (PSUM accumulator banks conflict with projection psum at the head, so
    PV lags one pair; exp outputs buffer in a deep pt pool).
  - The LN tail is per-t-chunk pipelined across ACT/PE/DVE with the sqrt
    table preloaded during pair-3 PV, and the output DMA split in quarters.
"""

import sys

for _p in ("/opt/trn_rl_repo", "/root/.axon_site/_ro/trn_rl_repo"):
    if _p not in sys.path:
        sys.path.insert(0, _p)

import numpy as np
import ml_dtypes

from concourse import bass, mybir
from concourse.tile import TileContext
from concourse.bass_utils import run_bass_kernel_spmd

B, C, T = 4, 512, 2048
H, D = 8, 64
SCALE = D ** -0.5
EPS = 1e-5
TQ = T // 2            # queries per core
KC = T // 128          # 16 key chunks
PAIRS = H // 2         # 4 head pairs
F32 = mybir.dt.float32
F32R = mybir.dt.float32r
BF16 = mybir.dt.bfloat16
bf16 = ml_dtypes.bfloat16

Exp = mybir.ActivationFunctionType.Exp
SCH_A = float(2**7 / np.log(2))    # Schraudolph fast-exp, bf16 bits in int16
SCH_B = float(127 * 2**7 - 7.5)
Sqrt = mybir.ActivationFunctionType.Sqrt
Ident = mybir.ActivationFunctionType.Identity
MULT = mybir.AluOpType.mult
ADD = mybir.AluOpType.add

_CACHE = {}


def _build_nc(reps=1, split_waits=True):
    nc = bass.Bass()
    xctm = nc.declare_dram_parameter("xctm", [128, 4 * T], BF16, False)
    xqm = nc.declare_dram_parameter("xqm", [128, 4 * TQ], BF16, False)
    xsm = nc.declare_dram_parameter("xsm", [128, 8 * C], BF16, False)
    wqm = nc.declare_dram_parameter("wqm", [128, 4 * C], BF16, False)
    wkm = nc.declare_dram_parameter("wkm", [128, 4 * C], BF16, False)
    wvm = nc.declare_dram_parameter("wvm", [128, 4 * C], BF16, False)
    wom = nc.declare_dram_parameter("wom", [128, 4 * C], BF16, False)
    cful = nc.declare_dram_parameter("cful", [128, KC], F32, False)
    c8 = nc.declare_dram_parameter("c8", [128, KC * H], BF16, False)
    gmm = nc.declare_dram_parameter("gmm", [128, 4], F32, False)
    bet = nc.declare_dram_parameter("bet", [128, 4], F32, False)
    iden = nc.declare_dram_parameter("iden", [128, 128], BF16, False)
    outp = nc.declare_dram_parameter("out", [128, 4 * TQ], BF16, True)

    with TileContext(nc) as tc:
        with (
            tc.sbuf_pool(name="cst", bufs=1) as cst,
            tc.sbuf_pool(name="pex", bufs=22) as pex,
            tc.sbuf_pool(name="sml", bufs=2) as sml,
            tc.psum_pool(name="ps", bufs=1) as ps,
        ):
            # ---- persistent state tiles ----
            epsT = cst.tile([128, 1], F32, name="epsT")
            dummy = cst.tile([128, 1], F32, name="dummy")
            KT = [cst.tile([128, T], BF16, name=f"KT{m}") for m in range(4)]
            QT = [cst.tile([128, TQ], BF16, name=f"QT{m}") for m in range(4)]
            VB = [cst.tile([128, H * 65], BF16, name=f"VB{k}") for k in range(KC)]
            OT = [cst.tile([128, TQ], BF16, name=f"OTp{p}") for p in range(PAIRS)]
            OUTS = cst.tile([128, 4 * TQ], BF16, name="OUTS")
            MV = cst.tile([128, 16], F32, name="MV")       # (mean, var) x 8 t
            RSD = cst.tile([128, 16], F32, name="RSD")     # rstd8 | -mu*rstd

            # preload the exp table while input DMAs run
            nc.vector.memset(epsT[:, :], EPS)
            nc.scalar.activation(dummy[:, :], epsT[:, :], Exp)

            # ---- input loads, compute-critical first ----
            WQb = cst.tile_from(wqm[:, :], name="WQb")
            XQb = cst.tile_from(xqm[:, :], name="XQb",
                                forced_dma_engine=mybir.EngineType.Pool)
            WKb = cst.tile_from(wkm[:, :], name="WKb")
            XCTb = cst.tile([128, 4 * T], BF16, name="XCTb")
            xctm3 = xctm.rearrange("p (ci t) -> p ci t", ci=4)
            xctb3 = XCTb.rearrange("p (ci t) -> p ci t", ci=4)
            for g in range(4):
                nc.sync.dma_start(out=xctb3[:, :, g * 512:(g + 1) * 512],
                                  in_=xctm3[:, :, g * 512:(g + 1) * 512])
            WVb = cst.tile_from(wvm[:, :], name="WVb")
            CF = cst.tile_from(cful[:, :], name="CF")
            C8 = cst.tile_from(c8[:, :], name="C8")
            ID = cst.tile_from(iden[:, :], name="ID")
            WOb = cst.tile_from(wom[:, :], name="WOb")
            XSb = cst.tile_from(xsm[:, :], name="XSb",
                                forced_dma_engine=mybir.EngineType.Pool)
            GM = cst.tile_from(gmm[:, :], name="GM")
            BT = cst.tile_from(bet[:, :], name="BT")
            WQ = [WQb[:, i * C:(i + 1) * C] for i in range(4)]
            XQ = [XQb[:, i * TQ:(i + 1) * TQ] for i in range(4)]
            WK = [WKb[:, i * C:(i + 1) * C] for i in range(4)]
            XCT = [XCTb[:, i * T:(i + 1) * T] for i in range(4)]
            WV = [WVb[:, i * C:(i + 1) * C] for i in range(4)]
            WO = [WOb[:, i * C:(i + 1) * C] for i in range(4)]
            XS = [XSb[:, t * C:(t + 1) * C] for t in range(8)]

            def _body(rp):
                # ---------- projection units (psum chunk each) ----------
                pcnt = [0]

                def q_unit(m, n):
                    qps = ps.tile([128, 512], F32,
                                  tag=("OA1", "OA2")[pcnt[0] % 2],
                                  name=f"qps{m}_{n}_r{rp}")
                    pcnt[0] += 1
                    for ci in range(4):
                        nc.tensor.matmul(
                            qps[:, :],
                            lhsT=WQ[ci][:, m * 128:(m + 1) * 128],
                            rhs=XQ[ci][:, n * 512:(n + 1) * 512],
                            start=(ci == 0), stop=(ci == 3))
                    nc.vector.tensor_copy(
                        QT[m][:, n * 512:(n + 1) * 512], qps[:, :])

                def k_unit(m, h2, n):
                    kps = ps.tile([128, 512], F32,
                                  tag=("OA1", "OA2")[pcnt[0] % 2],
                                  name=f"kps{m}_{h2}_{n}_r{rp}")
                    pcnt[0] += 1
                    for ci in range(4):
                        nc.tensor.matmul(
                            kps[:, :],
                            lhsT=WK[ci][:, m * 128:(m + 1) * 128],
                            rhs=XCT[ci][:, h2 * 1024 + n * 512:
                                        h2 * 1024 + (n + 1) * 512],
                            start=(ci == 0), stop=(ci == 3))
                    nc.vector.tensor_copy(
                        KT[m][:, h2 * 1024 + n * 512:h2 * 1024 + (n + 1) * 512],
                        kps[:, :])

                def v_unit(k):
                    vps = ps.tile([128, 512], F32, tag=("OB1", "OB2")[k % 2],
                                  name=f"vps{k}_r{rp}")
                    for ci in range(4):
                        nc.tensor.matmul(
                            vps[:, :],
                            lhsT=XCT[ci][:, k * 128:(k + 1) * 128],
                            rhs=WV[ci][:, :],
                            start=(ci == 0), stop=(ci == 3))
                    nc.vector.tensor_scalar(
                        out=VB[k].rearrange("p (h e) -> p h e", e=65)[:, :, 0:64],
                        in0=vps.rearrange("p (h e) -> p h e", e=64),
                        scalar1=CF[:, k:k + 1], scalar2=None, op0=MULT)
                    nc.vector.tensor_copy(
                        VB[k].rearrange("p (h e) -> p h e", e=65)[:, :, 64:65],
                        C8[:, k * H:(k + 1) * H].rearrange("p (h e) -> p h e", e=1))

                units = []
                units += [lambda n=n: k_unit(0, 1, n) for n in range(2)]
                units += [lambda n=n: q_unit(1, n) for n in range(2)]
                units += [lambda a=a: k_unit(1, *a)
                          for a in ((0, 0), (0, 1), (1, 0), (1, 1))]
                units += [lambda k=k: v_unit(k) for k in range(KC)]
                units += [lambda n=n: q_unit(2, n) for n in range(2)]
                units += [lambda a=a: k_unit(2, *a)
                          for a in ((0, 0), (0, 1), (1, 0), (1, 1))]
                units += [lambda n=n: q_unit(3, n) for n in range(2)]
                units += [lambda a=a: k_unit(3, *a)
                          for a in ((0, 0), (0, 1), (1, 0), (1, 1))]
                uidx = [0]

                def emit_units(cnt):
                    for _ in range(cnt):
                        if uidx[0] < len(units):
                            units[uidx[0]]()
                            uidx[0] += 1

                # upfront: QT0 fully, KT0 first half (covers S k<8)
                q_unit(0, 0); q_unit(0, 1); k_unit(0, 0, 0); k_unit(0, 0, 1)

                PT = {}    # (p, k, hi) -> pt tile, buffered until PV consumes
                last_pt = [None]

                def s_exp(p, k, hi, on_dve=False):
                    stag, ptag = ("SA", "pA") if hi == 0 else ("SB", "pB")
                    rows = slice(hi * 64, (hi + 1) * 64)
                    s_ps = ps.tile([128, 1024], F32, tag=stag,
                                   name=f"s{p}_{hi}_{k}_r{rp}")
                    for n in range(2):
                        nc.tensor.matmul(
                            s_ps[:, n * 512:(n + 1) * 512],
                            lhsT=KT[p][rows, k * 128:(k + 1) * 128],
                            rhs=QT[p][rows, n * 512:(n + 1) * 512],
                            start=True, stop=True)
                    pt = pex.tile([128, 1024], BF16, tag=ptag,
                                  name=f"pt{p}_{hi}_{k}_r{rp}")
                    if on_dve:
                        nc.vector.tensor_scalar(
                            out=pt.bitcast(mybir.dt.int16)[:, :], in0=s_ps[:, :],
                            scalar1=SCH_A, scalar2=SCH_B, op0=MULT, op1=ADD)
                    else:
                        nc.scalar.activation(pt[:, :], s_ps[:, :], Exp)
                        last_pt[0] = pt
                    PT[(p, k, hi)] = pt

                OACC = {}  # p -> {hi: [2 psum tiles]}

                def pv_half(p, k, hi):
                    if p not in OACC:
                        OACC[p] = {
                            0: [ps.tile([128, 512], F32, tag=("OA1", "OA2")[bk],
                                        name=f"o{p}_0_{bk}_r{rp}")
                                for bk in range(2)],
                            1: [ps.tile([128, 512], F32, tag=("OB1", "OB2")[bk],
                                        name=f"o{p}_1_{bk}_r{rp}")
                                for bk in range(2)],
                        }
                    pt = PT.pop((p, k, hi))
                    head = 2 * p + hi
                    for s in range(8):
                        bk, j = s // 4, s % 4
                        nc.tensor.matmul(
                            OACC[p][hi][bk][:, j * 65:(j + 1) * 65],
                            lhsT=pt[:, s * 128:(s + 1) * 128],
                            rhs=VB[k][:, head * 65:(head + 1) * 65],
                            start=(k == 0), stop=(k == KC - 1),
                            skip_group_check=True)

                def pv_chunk(p, k):
                    pv_half(p, k, 0)
                    pv_half(p, k, 1)

                def epilogue(p, use_act=False):
                    oacc = OACC.pop(p)
                    nmul = 0
                    ONs = [sml.tile([128, 128], BF16, tag="on",
                                    name=f"on{p}_{s}_r{rp}", bufs=10)
                           for s in range(8)]
                    for hi in (0, 1):
                        for bk in range(2):
                            o_ps = oacc[hi][bk]
                            ov = o_ps[:, 0:260].rearrange("p (s e) -> p s e", e=65)
                            rd4 = sml.tile([128, 4], F32, tag="rd",
                                           name=f"rd{p}_{hi}_{bk}_r{rp}", bufs=4)
                            nc.vector.reciprocal(
                                rd4.rearrange("p (s e) -> p s e", e=1),
                                ov[:, :, 64:65])
                            for j in range(4):
                                s = bk * 4 + j
                                nmul += 1
                                if use_act and nmul > 10:
                                    nc.scalar.activation(
                                        ONs[s][:, hi * 64:(hi + 1) * 64],
                                        o_ps[:, j * 65:j * 65 + 64],
                                        Ident, scale=rd4[:, j:j + 1])
                                else:
                                    nc.vector.tensor_scalar_mul(
                                        ONs[s][:, hi * 64:(hi + 1) * 64],
                                        o_ps[:, j * 65:j * 65 + 64],
                                        rd4[:, j:j + 1])
                    tps = [ps.tile([128, 512], BF16, tag=("OA1", "OA2")[b_],
                                   name=f"tp{p}_{b_}_r{rp}")
                           for b_ in range(2)]
                    for s in range(8):
                        nc.tensor.transpose(
                            tps[s // 4][:, (s % 4) * 128:(s % 4 + 1) * 128],
                            ONs[s][:, :], ID[:, :])
                    for b_ in range(2):
                        nc.vector.tensor_copy(OT[p][:, b_ * 512:(b_ + 1) * 512],
                                              tps[b_][:, :])

                # ---------- pair loops: S/exp for p, PV for p-1 ----------
                for p in range(PAIRS):
                    for k in range(KC):
                        s_exp(p, k, 0, on_dve=(p > 0 and k % 2 == 1))
                        if p > 0 and k > 0:
                            pv_half(p - 1, k - 1, 0)
                        s_exp(p, k, 1, on_dve=(p > 0 and k % 2 == 0))
                        if p == 0:
                            emit_units(1 + (k % 2) + (1 if k >= 8 else 0))
                        elif k > 0:
                            pv_half(p - 1, k - 1, 1)
                    emit_units(99)  # flush leftovers (end of pair 0 only)
                    if p > 0:
                        pv_chunk(p - 1, KC - 1)
                        epilogue(p - 1, use_act=True)

                # preload sqrt table right after the last exp (the read of
                # last_pt pins this behind the final Exp so the scheduler
                # cannot hoist it ahead and thrash the exp table)
                nc.scalar.activation(dummy[:, :], last_pt[0][:, 0:1], Sqrt)

                # ---------- PV(3) + early out_proj partials (t<4) ----------
                opsT = [None] * 8
                bigs = []
                for i in range(2):
                    big = ps.tile([128, 1024], F32, tag=("SA", "SB")[i],
                                  name=f"opb{i}_r{rp}")
                    bigs.append(big)
                    opsT[2 * i] = big[:, 0:512]
                    opsT[2 * i + 1] = big[:, 512:1024]
                for k in range(KC):
                    pv_chunk(3, k)
                    if k < 4:
                        t = k
                        for p_ in range(3):
                            nc.tensor.matmul(
                                opsT[t][:, :],
                                lhsT=OT[p_][:, t * 128:(t + 1) * 128],
                                rhs=WO[p_][:, :],
                                start=(p_ == 0), stop=False)
                epilogue(3, use_act=True)

                # ---------- finish out_proj + residual + LN stats ----------
                for t in range(8):
                    if t >= 4:
                        opsT[t] = ps.tile([128, 512], F32,
                                          tag=("OA1", "OA2", "OB1", "OB2")[t - 4],
                                          name=f"op{t}_r{rp}")
                        for p_ in range(4):
                            nc.tensor.matmul(
                                opsT[t][:, :],
                                lhsT=OT[p_][:, t * 128:(t + 1) * 128],
                                rhs=WO[p_][:, :],
                                start=(p_ == 0), stop=False)
                    else:
                        nc.tensor.matmul(
                            opsT[t][:, :],
                            lhsT=OT[3][:, t * 128:(t + 1) * 128],
                            rhs=WO[3][:, :],
                            start=False, stop=False)
                    nc.tensor.matmul(
                        opsT[t][:, :], lhsT=ID[:, :], rhs=XS[t][:, :],
                        start=False, stop=True)
                    bnst = sml.tile([128, 6], F32, tag="bnst",
                                    name=f"bnst{t}_r{rp}", bufs=3)
                    nc.vector.bn_stats(bnst[:, :], opsT[t][:, :])
                    nc.vector.bn_aggr(MV[:, 2 * t:2 * t + 2], bnst[:, :])

                # ---------- per-t pipelined LN + transpose + store ----------
                HN = [sml.tile([128, C], BF16, tag="hn", name=f"hn{t}_r{rp}",
                               bufs=8) for t in range(8)]
                std = [sml.tile([128, 1], F32, tag="std", name=f"std{t}_r{rp}",
                                bufs=4) for t in range(8)]

                def ln_t(t):
                    nc.scalar.activation(std[t][:, :], MV[:, 2 * t + 1:2 * t + 2],
                                         Sqrt, bias=epsT[:, :])
                    nc.vector.reciprocal(RSD[:, t:t + 1], std[t][:, :])
                    nc.vector.tensor_scalar(
                        out=RSD[:, 8 + t:9 + t], in0=MV[:, 2 * t:2 * t + 1],
                        scalar1=RSD[:, t:t + 1], scalar2=-1.0,
                        op0=MULT, op1=MULT)
                    nc.scalar.activation(HN[t][:, :], opsT[t][:, :], Ident,
                                         scale=RSD[:, t:t + 1],
                                         bias=RSD[:, 8 + t:9 + t])

                def ftp_t(t, tag):
                    ftp = ps.tile([128, 512], BF16, tag=tag, name=f"ftp{t}_r{rp}")
                    for cc in range(4):
                        nc.tensor.transpose(
                            ftp[:, cc * 128:(cc + 1) * 128],
                            HN[t][:, cc * 128:(cc + 1) * 128], ID[:, :])
                    for cc in range(4):
                        dst = OUTS[:, t * 512 + cc * 128:t * 512 + (cc + 1) * 128]
                        srcv = ftp[:, cc * 128:(cc + 1) * 128]
                        if cc == 3:
                            nc.scalar.activation(dst, srcv, Ident,
                                                 scale=GM[:, cc:cc + 1],
                                                 bias=BT[:, cc:cc + 1])
                        else:
                            nc.vector.tensor_scalar(
                                out=dst, in0=srcv,
                                scalar1=GM[:, cc:cc + 1], scalar2=BT[:, cc:cc + 1],
                                op0=MULT, op1=ADD)

                ln_t(0); ln_t(1); ln_t(2); ln_t(3)
                ftp_t(0, "SA"); ftp_t(1, "SA")
                ftp_t(2, "SB"); ftp_t(3, "SB")
                nc.sync.dma_start(out=outp[:, 0:2048], in_=OUTS[:, 0:2048])
                for t in range(4, 8):
                    ln_t(t)
                    ftp_t(t, ("OA1", "OA2", "OB1", "OB2")[t - 4])
                    if t % 2 == 1:
                        nc.sync.dma_start(
                            out=outp[:, (t - 1) * 512:(t + 1) * 512],
                            in_=OUTS[:, (t - 1) * 512:(t + 1) * 512])

            for rp in range(reps):
                _body(rp)

    if split_waits:
        _split_mm_waits(nc)
    return nc


def _split_mm_waits(nc):
    """Walrus MM structs carry only one sync wait; move extras to a NoOp."""
    f = nc.m.functions[0]
    for bb in f.blocks:
        il = bb.instructions
        out, changed = [], False
        for i in il:
            si = getattr(i, "sync_info", None)
            tn = type(i).__name__
            splittable = tn.startswith("Inst") and tn not in ("InstNoOp", "InstAllEngineBarrier")
            if (splittable and si is not None
                    and si.on_wait is not None and len(si.on_wait) > 1):
                waits = list(si.on_wait)
                for wi, w in enumerate(waits[:-1]):
                    out.append(mybir.InstNoOp(
                        name=f"{i.name}-wsplit{wi}", engine=i.engine,
                        sync_info=mybir.SyncInfo(on_wait=[w], on_update=[])))
                i.sync_info = mybir.SyncInfo(
                    on_wait=[waits[-1]], on_update=list(si.on_update))
                changed = True
            out.append(i)
        if changed:
            bb.instructions = out


def _prep_inputs(x, sqi, w_qkv, w_out, b_out, w_conv, b_conv, ln_gamma, ln_beta):
    x = np.asarray(x, np.float32)
    sqi = np.asarray(sqi, np.float32)
    w_qkv = np.asarray(w_qkv, np.float32)
    w_out = np.asarray(w_out, np.float32)
    b_out = np.asarray(b_out, np.float32)
    w_conv = np.asarray(w_conv, np.float32)
    b_conv = np.asarray(b_conv, np.float32)
    ln_gamma = np.asarray(ln_gamma, np.float32)
    ln_beta = np.asarray(ln_beta, np.float32)

    sp = np.pad(sqi, ((0, 0), (1, 1)))
    bias = (w_conv[0] * sp[:, :-2] + w_conv[1] * sp[:, 1:-1]
            + w_conv[2] * sp[:, 2:] + b_conv)                    # (B, T)
    c = np.exp(bias).astype(np.float32)

    def wchunks(wT):
        return np.ascontiguousarray(
            wT.reshape(4, 128, C).transpose(1, 0, 2).reshape(128, 4 * C)
        ).astype(bf16)

    wqT = (w_qkv[:C].T * SCALE).astype(np.float32)
    wkT = w_qkv[C:2 * C].T.astype(np.float32)
    wvT = w_qkv[2 * C:].T.astype(np.float32)
    woT = w_out.T.astype(np.float32)
    wqm, wkm, wvm, wom = (wchunks(w) for w in (wqT, wkT, wvT, woT))
    gm = ln_gamma.reshape(4, 128).T.copy().astype(np.float32)
    bt = ln_beta.reshape(4, 128).T.copy().astype(np.float32)
    iden = np.eye(128, dtype=bf16)

    in_maps = []
    for core in range(8):
        b, qh = divmod(core, 2)
        qs = slice(qh * TQ, (qh + 1) * TQ)
        cb = c[b]
        cful = cb.reshape(KC, 128).T.copy().astype(np.float32)
        c8 = np.repeat(cb.reshape(KC, 128).T, H, axis=1).copy().astype(bf16)
        xb = x[b].astype(bf16)
        xctm = np.ascontiguousarray(
            xb.reshape(4, 128, T).transpose(1, 0, 2).reshape(128, 4 * T))
        xqm = np.ascontiguousarray(
            xb[:, qs].reshape(4, 128, TQ).transpose(1, 0, 2).reshape(128, 4 * TQ))
        xs = (x[b].T[qs] + b_out).astype(np.float32)            # (TQ, C)
        xsm = np.ascontiguousarray(
            xs.reshape(8, 128, C).transpose(1, 0, 2).reshape(128, 8 * C)
        ).astype(bf16)
        in_maps.append({
            "xctm": xctm, "xqm": xqm, "xsm": xsm,
            "wqm": wqm, "wkm": wkm, "wvm": wvm, "wom": wom,
            "cful": cful, "c8": c8, "gmm": gm, "bet": bt,
"iden": iden,
        })
    return in_maps


def _unpack_out(o):
    """[128, t*512 + cc*128 + j] bf16 -> (C, TQ) f32."""
    o = np.asarray(o, dtype=np.float32).reshape(128, 8, 4, 128)
    return np.ascontiguousarray(o.transpose(2, 0, 1, 3)).reshape(C, TQ)


def kernel(x, sqi, w_qkv, w_out, b_out, w_conv, b_conv, ln_gamma, ln_beta,
           _trace=False):
    if "nc" not in _CACHE:
        _CACHE["nc"] = _build_nc()
    nc = _CACHE["nc"]
    in_maps = _prep_inputs(x, sqi, w_qkv, w_out, b_out, w_conv, b_conv,
                           ln_gamma, ln_beta)
    res = run_bass_kernel_spmd(nc, in_maps, core_ids=list(range(8)), trace=_trace)
    _CACHE["last_result"] = res
    out = np.empty((B, C, T), np.float32)
    for core in range(8):
        b, qh = divmod(core, 2)
        out[b][:, qh * TQ:(qh + 1) * TQ] = _unpack_out(res.results[core]["out"])
    return out


# revision 47
# speedup vs baseline: 1.0723x; 1.0089x over previous
"""Trainium2 Bass kernel for LogitBiasedSelfAttention1D.

Sharding: 8 cores = (batch b in 0..3) x (query half qh in 0..1).
Each core computes full attention (all 8 heads, all 2048 keys) for its
1024 queries of its batch. No collectives.

Math decomposition (exactly equivalent to the reference up to fp):
  - conv1d key bias folded into V:  softmax(S + bias) @ V
      = (exp(S) @ (c * V)) / (exp(S) @ c),   c = exp(bias)
  - SCALE folded into w_q on host.
  - residual x_seq + b_out added via identity matmul (f32r) into the
    out_proj PSUM accumulation.
  - LayerNorm normalize runs on ScalarE (Identity with scale/bias APs).

Schedule: the exp softmax on ScalarE is the bottleneck (~141us busy), so
everything is pipelined around keeping it fed:
  - QKV projections are emitted as small psum-chunk units interleaved into
    head-pair 0's S-loop (separate PSUM tags), so exp starts ~6us in.
  - PV (attn @ V) for pair p runs interleaved inside pair p+1's S-loop
    (PSUM accumulator banks conflict with projection psum at the head, so
    PV lags one pair; exp outputs buffer in a deep pt pool).
  - The LN tail is per-t-chunk pipelined across ACT/PE/DVE with the sqrt
    table preloaded during pair-3 PV, and the output DMA split in quarters.
"""

import sys

for _p in ("/opt/trn_rl_repo", "/root/.axon_site/_ro/trn_rl_repo"):
    if _p not in sys.path:
        sys.path.insert(0, _p)

import numpy as np
import ml_dtypes

from concourse import bass, mybir
from concourse.tile import TileContext
from concourse.bass_utils import run_bass_kernel_spmd

B, C, T = 4, 512, 2048
H, D = 8, 64
SCALE = D ** -0.5
EPS = 1e-5
TQ = T // 2            # queries per core
KC = T // 128          # 16 key chunks
PAIRS = H // 2         # 4 head pairs
F32 = mybir.dt.float32
F32R = mybir.dt.float32r
BF16 = mybir.dt.bfloat16
bf16 = ml_dtypes.bfloat16

Exp = mybir.ActivationFunctionType.Exp
SCH_A = float(2**7 / np.log(2))    # Schraudolph fast-exp, bf16 bits in int16
SCH_B = float(127 * 2**7 - 7.5)
Sqrt = mybir.ActivationFunctionType.Sqrt
Ident = mybir.ActivationFunctionType.Identity
MULT = mybir.AluOpType.mult
ADD = mybir.AluOpType.add

_CACHE = {}


def _build_nc(reps=1, split_waits=True):
    nc = bass.Bass()
    xctm = nc.declare_dram_parameter("xctm", [128, 4 * T], BF16, False)
    xqm = nc.declare_dram_parameter("xqm", [128, 4 * TQ], BF16, False)
    xsm = nc.declare_dram_parameter("xsm", [128, 8 * C], BF16, False)
    wqm = nc.declare_dram_parameter("wqm", [128, 4 * C], BF16, False)
    wkm = nc.declare_dram_parameter("wkm", [128, 4 * C], BF16, False)
    wvm = nc.declare_dram_parameter("wvm", [128, 4 * C], BF16, False)
    wom = nc.declare_dram_parameter("wom", [128, 4 * C], BF16, False)
    cful = nc.declare_dram_parameter("cful", [128, KC], F32, False)
    c8 = nc.declare_dram_parameter("c8", [128, KC * H], BF16, False)
    gmm = nc.declare_dram_parameter("gmm", [128, 4], F32, False)
    bet = nc.declare_dram_parameter("bet", [128, 4], F32, False)
    iden = nc.declare_dram_parameter("iden", [128, 128], BF16, False)
    outp = nc.declare_dram_parameter("out", [128, 4 * TQ], BF16, True)

    with TileContext(nc) as tc:
        with (
            tc.sbuf_pool(name="cst", bufs=1) as cst,
            tc.sbuf_pool(name="pex", bufs=22) as pex,
            tc.sbuf_pool(name="sml", bufs=2) as sml,
            tc.psum_pool(name="ps", bufs=1) as ps,
        ):
            # ---- persistent state tiles ----
            epsT = cst.tile([128, 1], F32, name="epsT")
            dummy = cst.tile([128, 1], F32, name="dummy")
            KT = [cst.tile([128, T], BF16, name=f"KT{m}") for m in range(4)]
            QT = [cst.tile([128, TQ], BF16, name=f"QT{m}") for m in range(4)]
            VB = [cst.tile([128, H * 65], BF16, name=f"VB{k}") for k in range(KC)]
            OT = [cst.tile([128, TQ], BF16, name=f"OTp{p}") for p in range(PAIRS)]
            OUTS = cst.tile([128, 4 * TQ], BF16, name="OUTS")
            MV = cst.tile([128, 16], F32, name="MV")       # (mean, var) x 8 t
            RSD = cst.tile([128, 16], F32, name="RSD")     # rstd8 | -mu*rstd

            # preload the exp table while input DMAs run
            nc.vector.memset(epsT[:, :], EPS)
            nc.scalar.activation(dummy[:, :], epsT[:, :], Exp)

            # ---- input loads, compute-critical first ----
            WQb = cst.tile_from(wqm[:, :], name="WQb")
            XQb = cst.tile_from(xqm[:, :], name="XQb",
                                forced_dma_engine=mybir.EngineType.Pool)
            WKb = cst.tile_from(wkm[:, :], name="WKb")
            XCTb = cst.tile([128, 4 * T], BF16, name="XCTb")
            xctm3 = xctm.rearrange("p (ci t) -> p ci t", ci=4)
            xctb3 = XCTb.rearrange("p (ci t) -> p ci t", ci=4)
            for g in range(4):
                nc.sync.dma_start(out=xctb3[:, :, g * 512:(g + 1) * 512],
                                  in_=xctm3[:, :, g * 512:(g + 1) * 512])
            WVb = cst.tile_from(wvm[:, :], name="WVb")
            CF = cst.tile_from(cful[:, :], name="CF")
            C8 = cst.tile_from(c8[:, :], name="C8")
            ID = cst.tile_from(iden[:, :], name="ID")
            WOb = cst.tile_from(wom[:, :], name="WOb")
            XSb = cst.tile_from(xsm[:, :], name="XSb",
                                forced_dma_engine=mybir.EngineType.Pool)
            GM = cst.tile_from(gmm[:, :], name="GM")
            BT = cst.tile_from(bet[:, :], name="BT")
            WQ = [WQb[:, i * C:(i + 1) * C] for i in range(4)]
            XQ = [XQb[:, i * TQ:(i + 1) * TQ] for i in range(4)]
            WK = [WKb[:, i * C:(i + 1) * C] for i in range(4)]
            XCT = [XCTb[:, i * T:(i + 1) * T] for i in range(4)]
            WV = [WVb[:, i * C:(i + 1) * C] for i in range(4)]
            WO = [WOb[:, i * C:(i + 1) * C] for i in range(4)]
            XS = [XSb[:, t * C:(t + 1) * C] for t in range(8)]

            def _body(rp):
                # ---------- projection units (psum chunk each) ----------
                pcnt = [0]

                def q_unit(m, n):
                    qps = ps.tile([128, 512], F32,
                                  tag=("OA1", "OA2")[pcnt[0] % 2],
                                  name=f"qps{m}_{n}_r{rp}")
                    pcnt[0] += 1
                    for ci in range(4):
                        nc.tensor.matmul(
                            qps[:, :],
                            lhsT=WQ[ci][:, m * 128:(m + 1) * 128],
                            rhs=XQ[ci][:, n * 512:(n + 1) * 512],
                            start=(ci == 0), stop=(ci == 3))
                    nc.vector.tensor_copy(
                        QT[m][:, n * 512:(n + 1) * 512], qps[:, :])

                def k_unit(m, h2, n):
                    kps = ps.tile([128, 512], F32,
                                  tag=("OA1", "OA2")[pcnt[0] % 2],
                                  name=f"kps{m}_{h2}_{n}_r{rp}")
                    pcnt[0] += 1
                    for ci in range(4):
                        nc.tensor.matmul(
                            kps[:, :],
                            lhsT=WK[ci][:, m * 128:(m + 1) * 128],
                            rhs=XCT[ci][:, h2 * 1024 + n * 512:
                                        h2 * 1024 + (n + 1) * 512],
                            start=(ci == 0), stop=(ci == 3))
                    nc.vector.tensor_copy(
                        KT[m][:, h2 * 1024 + n * 512:h2 * 1024 + (n + 1) * 512],
                        kps[:, :])

                def v_unit(k):
                    vps = ps.tile([128, 512], F32, tag=("OB1", "OB2")[k % 2],
                                  name=f"vps{k}_r{rp}")
                    for ci in range(4):
                        nc.tensor.matmul(
                            vps[:, :],
                            lhsT=XCT[ci][:, k * 128:(k + 1) * 128],
                            rhs=WV[ci][:, :],
                            start=(ci == 0), stop=(ci == 3))
                    nc.vector.tensor_scalar(
                        out=VB[k].rearrange("p (h e) -> p h e", e=65)[:, :, 0:64],
                        in0=vps.rearrange("p (h e) -> p h e", e=64),
                        scalar1=CF[:, k:k + 1], scalar2=None, op0=MULT)
                    nc.vector.tensor_copy(
                        VB[k].rearrange("p (h e) -> p h e", e=65)[:, :, 64:65],
                        C8[:, k * H:(k + 1) * H].rearrange("p (h e) -> p h e", e=1))

                units = []
                units += [lambda n=n: k_unit(0, 1, n) for n in range(2)]
                units += [lambda n=n: q_unit(1, n) for n in range(2)]
                units += [lambda a=a: k_unit(1, *a)
                          for a in ((0, 0), (0, 1), (1, 0), (1, 1))]
                units += [lambda k=k: v_unit(k) for k in range(KC)]
                units += [lambda n=n: q_unit(2, n) for n in range(2)]
                units += [lambda a=a: k_unit(2, *a)
                          for a in ((0, 0), (0, 1), (1, 0), (1, 1))]
                units += [lambda n=n: q_unit(3, n) for n in range(2)]
                units += [lambda a=a: k_unit(3, *a)
                          for a in ((0, 0), (0, 1), (1, 0), (1, 1))]
                uidx = [0]

                def emit_units(cnt):
                    for _ in range(cnt):
                        if uidx[0] < len(units):
                            units[uidx[0]]()
                            uidx[0] += 1

                # upfront: QT0 fully, KT0 first half (covers S k<8)
                q_unit(0, 0); q_unit(0, 1); k_unit(0, 0, 0); k_unit(0, 0, 1)

                PT = {}    # (p, k, hi) -> pt tile, buffered until PV consumes
                last_pt = [None]

                def s_exp(p, k, hi, on_dve=False):
                    stag, ptag = ("SA", "pA") if hi == 0 else ("SB", "pB")
                    rows = slice(hi * 64, (hi + 1) * 64)
                    s_ps = ps.tile([128, 1024], F32, tag=stag,
                                   name=f"s{p}_{hi}_{k}_r{rp}")
                    for n in range(2):
                        nc.tensor.matmul(
                            s_ps[:, n * 512:(n + 1) * 512],
                            lhsT=KT[p][rows, k * 128:(k + 1) * 128],
                            rhs=QT[p][rows, n * 512:(n + 1) * 512],
                            start=True, stop=True)
                    pt = pex.tile([128, 1024], BF16, tag=ptag,
                                  name=f"pt{p}_{hi}_{k}_r{rp}")
                    if on_dve:
                        nc.vector.tensor_scalar(
                            out=pt.bitcast(mybir.dt.int16)[:, :], in0=s_ps[:, :],
                            scalar1=SCH_A, scalar2=SCH_B, op0=MULT, op1=ADD)
                    else:
                        nc.scalar.activation(pt[:, :], s_ps[:, :], Exp)
                        last_pt[0] = pt
                    PT[(p, k, hi)] = pt

                OACC = {}  # p -> {hi: [2 psum tiles]}

                def pv_half(p, k, hi):
                    if p not in OACC:
                        OACC[p] = {
                            0: [ps.tile([128, 512], F32, tag=("OA1", "OA2")[bk],
                                        name=f"o{p}_0_{bk}_r{rp}")
                                for bk in range(2)],
                            1: [ps.tile([128, 512], F32, tag=("OB1", "OB2")[bk],
                                        name=f"o{p}_1_{bk}_r{rp}")
                                for bk in range(2)],
                        }
                    pt = PT.pop((p, k, hi))
                    head = 2 * p + hi
                    for s in range(8):
                        bk, j = s // 4, s % 4
                        nc.tensor.matmul(
                            OACC[p][hi][bk][:, j * 65:(j + 1) * 65],
                            lhsT=pt[:, s * 128:(s + 1) * 128],
                            rhs=VB[k][:, head * 65:(head + 1) * 65],
                            start=(k == 0), stop=(k == KC - 1),
                            skip_group_check=True)

                def pv_chunk(p, k):
                    pv_half(p, k, 0)
                    pv_half(p, k, 1)

                def epilogue(p, use_act=False):
                    oacc = OACC.pop(p)
                    nmul = 0
                    ONs = [sml.tile([128, 128], BF16, tag="on",
                                    name=f"on{p}_{s}_r{rp}", bufs=10)
                           for s in range(8)]
                    for hi in (0, 1):
                        for bk in range(2):
                            o_ps = oacc[hi][bk]
                            ov = o_ps[:, 0:260].rearrange("p (s e) -> p s e", e=65)
                            rd4 = sml.tile([128, 4], F32, tag="rd",
                                           name=f"rd{p}_{hi}_{bk}_r{rp}", bufs=4)
                            nc.vector.reciprocal(
                                rd4.rearrange("p (s e) -> p s e", e=1),
                                ov[:, :, 64:65])
                            for j in range(4):
                                s = bk * 4 + j
                                nmul += 1
                                if use_act and nmul > 10:
                                    nc.scalar.activation(
                                        ONs[s][:, hi * 64:(hi + 1) * 64],
                                        o_ps[:, j * 65:j * 65 + 64],
                                        Ident, scale=rd4[:, j:j + 1])
                                else:
                                    nc.vector.tensor_scalar_mul(
                                        ONs[s][:, hi * 64:(hi + 1) * 64],
                                        o_ps[:, j * 65:j * 65 + 64],
                                        rd4[:, j:j + 1])
                    tps = [ps.tile([128, 512], BF16, tag=("OA1", "OA2")[b_],
                                   name=f"tp{p}_{b_}_r{rp}")
                           for b_ in range(2)]
                    for s in range(8):
                        nc.tensor.transpose(
                            tps[s // 4][:, (s % 4) * 128:(s % 4 + 1) * 128],
                            ONs[s][:, :], ID[:, :])
                    for b_ in range(2):
                        nc.scalar.activation(OT[p][:, b_ * 512:(b_ + 1) * 512],
                                             tps[b_][:, :], Ident)

                # ---------- pair loops: S/exp for p, PV for p-1 ----------
                for p in range(PAIRS):
                    for k in range(KC):
                        s_exp(p, k, 0, on_dve=(p > 0 and k % 2 == 1))
                        if p > 0 and k > 0:
                            pv_half(p - 1, k - 1, 0)
                        s_exp(p, k, 1, on_dve=(p > 0 and k % 2 == 0 and k > 0))
                        if p == 0:
                            emit_units(1 + (k % 2) + (1 if k >= 8 else 0))
                        elif k > 0:
                            pv_half(p - 1, k - 1, 1)
                    emit_units(99)  # flush leftovers (end of pair 0 only)
                    if p > 0:
                        pv_chunk(p - 1, KC - 1)
                        epilogue(p - 1, use_act=True)

                # preload sqrt table right after the last exp (the read of
                # last_pt pins this behind the final Exp so the scheduler
                # cannot hoist it ahead and thrash the exp table)
                nc.scalar.activation(dummy[:, :], last_pt[0][:, 0:1], Sqrt)

                # ---------- PV(3) + early out_proj partials (t<4) ----------
                opsT = [None] * 8
                bigs = []
                for i in range(2):
                    big = ps.tile([128, 1024], F32, tag=("SA", "SB")[i],
                                  name=f"opb{i}_r{rp}")
                    bigs.append(big)
                    opsT[2 * i] = big[:, 0:512]
                    opsT[2 * i + 1] = big[:, 512:1024]
                for k in range(KC):
                    pv_chunk(3, k)
                    if k < 4:
                        t = k
                        for p_ in range(3):
                            nc.tensor.matmul(
                                opsT[t][:, :],
                                lhsT=OT[p_][:, t * 128:(t + 1) * 128],
                                rhs=WO[p_][:, :],
                                start=(p_ == 0), stop=False)
                epilogue(3, use_act=True)

                # ---------- finish out_proj + residual + LN stats ----------
                for t in range(8):
                    if t >= 4:
                        opsT[t] = ps.tile([128, 512], F32,
                                          tag=("OA1", "OA2", "OB1", "OB2")[t - 4],
                                          name=f"op{t}_r{rp}")
                        for p_ in range(4):
                            nc.tensor.matmul(
                                opsT[t][:, :],
                                lhsT=OT[p_][:, t * 128:(t + 1) * 128],
                                rhs=WO[p_][:, :],
                                start=(p_ == 0), stop=False)
                    else:
                        nc.tensor.matmul(
                            opsT[t][:, :],
                            lhsT=OT[3][:, t * 128:(t + 1) * 128],
                            rhs=WO[3][:, :],
                            start=False, stop=False)
                    nc.tensor.matmul(
                        opsT[t][:, :], lhsT=ID[:, :], rhs=XS[t][:, :],
                        start=False, stop=True)
                    bnst = sml.tile([128, 6], F32, tag="bnst",
                                    name=f"bnst{t}_r{rp}", bufs=3)
                    nc.vector.bn_stats(bnst[:, :], opsT[t][:, :])
                    nc.vector.bn_aggr(MV[:, 2 * t:2 * t + 2], bnst[:, :])

                # ---------- per-t pipelined LN + transpose + store ----------
                HN = [sml.tile([128, C], BF16, tag="hn", name=f"hn{t}_r{rp}",
                               bufs=8) for t in range(8)]
                std = [sml.tile([128, 1], F32, tag="std", name=f"std{t}_r{rp}",
                                bufs=4) for t in range(8)]

                def ln_t(t):
                    nc.scalar.activation(std[t][:, :], MV[:, 2 * t + 1:2 * t + 2],
                                         Sqrt, bias=epsT[:, :])
                    nc.vector.reciprocal(RSD[:, t:t + 1], std[t][:, :])
                    nc.vector.tensor_scalar(
                        out=RSD[:, 8 + t:9 + t], in0=MV[:, 2 * t:2 * t + 1],
                        scalar1=RSD[:, t:t + 1], scalar2=-1.0,
                        op0=MULT, op1=MULT)
                    nc.scalar.activation(HN[t][:, :], opsT[t][:, :], Ident,
                                         scale=RSD[:, t:t + 1],
                                         bias=RSD[:, 8 + t:9 + t])

                def ftp_t(t, tag):
                    ftp = ps.tile([128, 512], BF16, tag=tag, name=f"ftp{t}_r{rp}")
                    for cc in range(4):
                        nc.tensor.transpose(
                            ftp[:, cc * 128:(cc + 1) * 128],
                            HN[t][:, cc * 128:(cc + 1) * 128], ID[:, :])
                    for cc in range(4):
                        dst = OUTS[:, t * 512 + cc * 128:t * 512 + (cc + 1) * 128]
                        srcv = ftp[:, cc * 128:(cc + 1) * 128]
                        if cc == 3:
                            nc.scalar.activation(dst, srcv, Ident,
                                                 scale=GM[:, cc:cc + 1],
                                                 bias=BT[:, cc:cc + 1])
                        else:
                            nc.vector.tensor_scalar(
                                out=dst, in0=srcv,
                                scalar1=GM[:, cc:cc + 1], scalar2=BT[:, cc:cc + 1],
                                op0=MULT, op1=ADD)

                ln_t(0); ln_t(1); ln_t(2); ln_t(3)
                ftp_t(0, "SA"); ftp_t(1, "SA")
                ftp_t(2, "SB"); ftp_t(3, "SB")
                nc.sync.dma_start(out=outp[:, 0:2048], in_=OUTS[:, 0:2048])
                for t in range(4, 8):
                    ln_t(t)
                    ftp_t(t, ("OA1", "OA2", "OB1", "OB2")[t - 4])
                    if t % 2 == 1:
                        nc.sync.dma_start(
                            out=outp[:, (t - 1) * 512:(t + 1) * 512],
                            in_=OUTS[:, (t - 1) * 512:(t + 1) * 512])

            for rp in range(reps):
                _body(rp)

    if split_waits:
        _split_mm_waits(nc)
    return nc


def _split_mm_waits(nc):
    """Walrus MM structs carry only one sync wait; move extras to a NoOp."""
    f = nc.m.functions[0]
    for bb in f.blocks:
        il = bb.instructions
        out, changed = [], False
        for i in il:
            si = getattr(i, "sync_info", None)
            tn = type(i).__name__
            splittable = tn.startswith("Inst") and tn not in ("InstNoOp", "InstAllEngineBarrier")
            if (splittable and si is not None
                    and si.on_wait is not None and len(si.on_wait) > 1):
                waits = list(si.on_wait)
                for wi, w in enumerate(waits[:-1]):
                    out.append(mybir.InstNoOp(
                        name=f"{i.name}-wsplit{wi}", engine=i.engine,
                        sync_info=mybir.SyncInfo(on_wait=[w], on_update=[])))
                i.sync_info = mybir.SyncInfo(
                    on_wait=[waits[-1]], on_update=list(si.on_update))
                changed = True
            out.append(i)
        if changed:
            bb.instructions = out


def _prep_inputs(x, sqi, w_qkv, w_out, b_out, w_conv, b_conv, ln_gamma, ln_beta):
    x = np.asarray(x, np.float32)
    sqi = np.asarray(sqi, np.float32)
    w_qkv = np.asarray(w_qkv, np.float32)
    w_out = np.asarray(w_out, np.float32)
    b_out = np.asarray(b_out, np.float32)
    w_conv = np.asarray(w_conv, np.float32)
    b_conv = np.asarray(b_conv, np.float32)
    ln_gamma = np.asarray(ln_gamma, np.float32)
    ln_beta = np.asarray(ln_beta, np.float32)

    sp = np.pad(sqi, ((0, 0), (1, 1)))
    bias = (w_conv[0] * sp[:, :-2] + w_conv[1] * sp[:, 1:-1]
            + w_conv[2] * sp[:, 2:] + b_conv)                    # (B, T)
    c = np.exp(bias).astype(np.float32)

    def wchunks(wT):
        return np.ascontiguousarray(
            wT.reshape(4, 128, C).transpose(1, 0, 2).reshape(128, 4 * C)
        ).astype(bf16)

    wqT = (w_qkv[:C].T * SCALE).astype(np.float32)
    wkT = w_qkv[C:2 * C].T.astype(np.float32)
    wvT = w_qkv[2 * C:].T.astype(np.float32)
    woT = w_out.T.astype(np.float32)
    wqm, wkm, wvm, wom = (wchunks(w) for w in (wqT, wkT, wvT, woT))
    gm = ln_gamma.reshape(4, 128).T.copy().astype(np.float32)
    bt = ln_beta.reshape(4, 128).T.copy().astype(np.float32)
    iden = np.eye(128, dtype=bf16)

    in_maps = []
    for core in range(8):
        b, qh = divmod(core, 2)
        qs = slice(qh * TQ, (qh + 1) * TQ)
        cb = c[b]
        cful = cb.reshape(KC, 128).T.copy().astype(np.float32)
        c8 = np.repeat(cb.reshape(KC, 128).T, H, axis=1).copy().astype(bf16)
        xb = x[b].astype(bf16)
        xctm = np.ascontiguousarray(
            xb.reshape(4, 128, T).transpose(1, 0, 2).reshape(128, 4 * T))
        xqm = np.ascontiguousarray(
            xb[:, qs].reshape(4, 128, TQ).transpose(1, 0, 2).reshape(128, 4 * TQ))
        xs = (x[b].T[qs] + b_out).astype(np.float32)            # (TQ, C)
        xsm = np.ascontiguousarray(
            xs.reshape(8, 128, C).transpose(1, 0, 2).reshape(128, 8 * C)
        ).astype(bf16)
        in_maps.append({
            "xctm": xctm, "xqm": xqm, "xsm": xsm,
            "wqm": wqm, "wkm": wkm, "wvm": wvm, "wom": wom,
            "cful": cful, "c8": c8, "gmm": gm, "bet": bt,
"iden": iden,
        })
    return in_maps


def _unpack_out(o):
    """[128, t*512 + cc*128 + j] bf16 -> (C, TQ) f32."""
    o = np.asarray(o, dtype=np.float32).reshape(128, 8, 4, 128)
    return np.ascontiguousarray(o.transpose(2, 0, 1, 3)).reshape(C, TQ)


def kernel(x, sqi, w_qkv, w_out, b_out, w_conv, b_conv, ln_gamma, ln_beta,
           _trace=False):
    if "nc" not in _CACHE:
        _CACHE["nc"] = _build_nc()
    nc = _CACHE["nc"]
    in_maps = _prep_inputs(x, sqi, w_qkv, w_out, b_out, w_conv, b_conv,
                           ln_gamma, ln_beta)
    res = run_bass_kernel_spmd(nc, in_maps, core_ids=list(range(8)), trace=_trace)
    _CACHE["last_result"] = res
    out = np.empty((B, C, T), np.float32)
    for core in range(8):
        b, qh = divmod(core, 2)
        out[b][:, qh * TQ:(qh + 1) * TQ] = _unpack_out(res.results[core]["out"])
    return out
